# revision 45
# baseline (speedup 1.0000x reference)
"""Trainium2 Bass kernel for nn_Net_NNCONV (gnn_message_passing).

Strategy (8-core SPMD, data-parallel by graph):
 - Host: cut graphs into 8 contiguous shards (balanced node counts); within a
   shard, bin-pack nodes into NB 128-node blocks with <=256 in-edges per block
   (TPB=2 edge tiles per block); edges grouped by dst block in slot order.
 - fp16 everywhere on device (PE fp16 = 1 cycle/row vs 4 for f32; DVE 2x mode
   needs 2-byte packed operands).
 - Phase 0: lin0 -> h0 fp16; AllGather h0; he = relu(ea@W1+b1) fp16 in SBUF.
 - Step 0 fuses the ew = he@W2 production (PE matmuls + ACT psum->fp16 casts)
   into the message block loop: the fp16 ew tile is consumed directly and also
   written to DRAM for steps 1-4, which stream it back.
 - Per block (256 edges): indirect-gather gout fp16, in-place broadcast
   multiply (DVE 2x), in-place halving-tree partial reduce 64->8 over i,
   S-matmul (fp16 one-hot, 1/deg folded) with rhs [64o,8i] into PSUM
   accumulating the block's 2 tiles, single 1x reduce psum->aggr fp16.
 - GRU: per-block PE transposes + gh/gi matmuls (fp16), ACT psum copies,
   gate math batched over all blocks as [128, NB*64] fp16 DVE ops.
 - Set2Set: per-node q via fp16 GT one-hot matmuls on the (idle) PE; e =
   rowsum(h * q) as batched DVE mult+reduce (tensor_tensor_reduce crashes the
   device); r_pool/asum via one-hot G matmuls (G fp16, SBUF-resident); softmax
   skips max-subtraction (args O(10), fp32 exp).
 - ew_sb is triple-buffered so the 2 MB/block fabric-limited DMA is fully
   prefetched; the block psum reduce + GRU stages are software-pipelined one
   block behind the S-matmuls so DVE never waits on PE.
 - lin0/he preamble is fused into the step-0 pipeline (shared PSUM pools, no
   barrier); GRU gate math for the first block-half is folded into the block
   loop so only half remains serial before each AllGather.
 - Set2Set: iteration 0 is algebraic (zero LSTM biases => q==0 => uniform
   softmax), so it reduces to a mean-pool; q-copies batched 8 blocks/copy;
   G loaded w-major on the ACT queue so iter-0 r_pool starts early.
 - Measured bounds (trn2): DVE fp16 tensor_tensor caps at 2 elem/cyc/lane
   (mult 4.3us + tree 3.7us per 256-edge block ~= 440us/step floor); Pool
   tensor_tensor is ~7.6x slower AND contends with DVE for SBUF ports (never
   offload elementwise there); multi-column indirect_dma_start crashes the
   device; Shared collective outputs allow only one writer instruction.
"""
import contextlib
import os
import sys

sys.path.insert(0, "/opt/trn_rl_repo")

import numpy as np

import concourse.bass as bass
import concourse.tile as tile
from concourse import bacc, mybir
from concourse.bass import IndirectOffsetOnAxis
from concourse.bass_utils import run_bass_kernel_spmd

F32 = mybir.dt.float32
F16 = mybir.dt.float16
I32 = mybir.dt.int32
NCORES = 8
P = 128
D = 64
DD = D * D
W = 128
TPB = 2
EPB = TPB * P          # max edges per block
STEPS = 5
Alu = mybir.AluOpType
Act = mybir.ActivationFunctionType


# ----------------------------------------------------------------------------
# Host-side preprocessing
# ----------------------------------------------------------------------------

def _pack_rank(deg, lo, hi, NB):
    """Assign nodes [lo,hi) to NB blocks: <=128 nodes, <=EPB in-edges each.
    Returns (block, pos) per node or None if packing fails."""
    nodes = np.arange(lo, hi)
    order = nodes[np.argsort(-deg[nodes], kind="stable")]
    loads = np.zeros(NB, dtype=np.int64)
    counts = np.zeros(NB, dtype=np.int64)
    blk = np.zeros(hi - lo, dtype=np.int64)
    for nd in order:
        d = deg[nd]
        cand = np.flatnonzero(counts < P)
        if len(cand) == 0:
            return None
        bi = cand[np.argmin(loads[cand])]
        if loads[bi] + d > EPB:
            return None
        blk[nd - lo] = bi
        loads[bi] += d
        counts[bi] += 1
    pos = np.zeros(hi - lo, dtype=np.int64)
    ctr = np.zeros(NB, dtype=np.int64)
    for nd in order:
        b = blk[nd - lo]
        pos[nd - lo] = ctr[b]
        ctr[b] += 1
    return blk, pos


def _preprocess(x, z, edge_attr, edge_index, batch, num_graphs):
    N = x.shape[0]
    E = edge_index.shape[1]
    B = int(num_graphs)
    src = np.asarray(edge_index[0], dtype=np.int64)
    dst = np.asarray(edge_index[1], dtype=np.int64)
    batch = np.asarray(batch, dtype=np.int64)

    # --- shard cut: contiguous graphs, balanced node counts ---
    gcounts = np.bincount(batch, minlength=B)
    gcum = np.concatenate([[0], np.cumsum(gcounts)])
    targets = (np.arange(1, NCORES) * N) / NCORES
    cuts = np.searchsorted(gcum, targets)
    gcut = np.concatenate([[0], cuts, [B]])
    n0 = gcum[gcut]
    nodes_per = np.diff(n0)
    graphs_per = np.diff(gcut)

    GW = int(np.ceil(graphs_per.max() / P))

    deg = np.bincount(dst, minlength=N).astype(np.int64)
    dinv = (1.0 / np.maximum(deg, 1)).astype(np.float32)
    e_rank = np.searchsorted(n0[1:], dst, side="right")
    edges_per = np.bincount(e_rank, minlength=NCORES)

    NB = int(max(np.ceil(nodes_per.max() / P), np.ceil(edges_per.max() / EPB)))
    packs = None
    while packs is None:
        packs = []
        for r in range(NCORES):
            res = _pack_rank(deg, int(n0[r]), int(n0[r + 1]), NB)
            if res is None:
                packs = None
                NB += 1
                break
            packs.append(res)

    ET = NB * TPB
    SN = NB * P
    SLOTS = ET * P

    # AllGather is emitted in two block-halves; gathered rows live at
    # [rank-major half-1 | rank-major half-2] in out_full.
    H1B = NB // 2
    H1 = H1B * P
    H2 = (NB - H1B) * P

    node_block = np.zeros(N, dtype=np.int64)
    node_pos = np.zeros(N, dtype=np.int64)
    pad_global = np.zeros(N, dtype=np.int64)
    ag_global = np.zeros(N, dtype=np.int64)
    for r in range(NCORES):
        lo, hi = int(n0[r]), int(n0[r + 1])
        blk, pos = packs[r]
        node_block[lo:hi] = blk
        node_pos[lo:hi] = pos
        pad_global[lo:hi] = r * SN + blk * P + pos
        ag_global[lo:hi] = r * SN + pos * NB + blk

    xa = np.concatenate([np.asarray(x, np.float32),
                         np.asarray(z, np.float32)[:, None],
                         np.ones((N, 1), np.float32)], axis=1)  # [N, 17]

    per_rank = []
    for r in range(NCORES):
        lo, hi = int(n0[r]), int(n0[r + 1])
        g0 = int(gcut[r])
        ngr = int(gcut[r + 1] - gcut[r])

        xaT = np.zeros((17, SN), np.float16)
        loc = pad_global[lo:hi] - r * SN
        xaT[:, loc] = xa[lo:hi].T.astype(np.float16)

        em = e_rank == r
        es, ed = src[em], dst[em]
        eb = node_block[ed]
        eorder = np.argsort(eb, kind="stable")
        es, ed, eb = es[eorder], ed[eorder], eb[eorder]
        ea_r = np.asarray(edge_attr, np.float32)[em][eorder]

        eaT = np.zeros((6, SLOTS), np.float16)
        src_idx = np.zeros(SLOTS, np.int32)
        S = np.zeros((ET, P, P), np.float16)
        bstart = np.searchsorted(eb, np.arange(NB))
        bend = np.searchsorted(eb, np.arange(NB), side="right")
        for b in range(NB):
            cnt = bend[b] - bstart[b]
            assert cnt <= EPB, f"rank {r} block {b} edges {cnt} > {EPB}"
            sl = slice(bstart[b], bend[b])
            base = b * EPB
            eaT[:5, base:base + cnt] = ea_r[sl].T.astype(np.float16)
            eaT[5, base:base + cnt] = 1.0
            src_idx[base:base + cnt] = ag_global[es[sl]].astype(np.int32)
            vcol = node_pos[ed[sl]].astype(np.int64)
            vals = dinv[ed[sl]]
            within = np.arange(cnt)
            S[b * TPB + within // P, within % P, vcol] = vals.astype(np.float16)

        idx2d = src_idx.reshape(ET, P).T.copy()  # [128, ET]

        # pooling one-hot G[(b,w)][node_p, g_col] (+ transpose for q lookup)
        G = np.zeros((NB * GW, P, P), np.float16)
        gl = batch[lo:hi] - g0
        bb = node_block[lo:hi]
        pp = node_pos[lo:hi]
        ww = gl // P
        cc_ = gl % P
        G[bb * GW + ww, pp, cc_] = 1.0
        GT = np.ascontiguousarray(np.transpose(G, (0, 2, 1)))

        per_rank.append(dict(xaT=xaT, eaT=eaT, idx=idx2d, S=S, G=G, GT=GT,
                             ngr=ngr))

    meta = dict(NB=NB, SN=SN, GW=GW, ET=ET, SLOTS=SLOTS, H1B=H1B)
    return per_rank, meta


def _weights(lin0_w, lin0_b, emlp_w1, emlp_b1, emlp_w2, emlp_b2,
             conv_root, conv_bias, gru_wi, gru_wh, gru_bi, gru_bh,
             lstm_wi, lstm_wh, lstm_bi, lstm_bh, lin1_w, lin1_b, lin2_w, lin2_b):
    f = lambda a: np.asarray(a, np.float32)
    h = lambda a: np.asarray(a, np.float16)
    W0b = h(np.concatenate([f(lin0_w), f(lin0_b)[None, :]], 0))            # [17, 64]
    W1b = h(np.concatenate([f(emlp_w1), f(emlp_b1)[None, :]], 0))          # [6, 128]
    W2 = f(emlp_w2)
    b2e = f(emlp_b2)
    W2p = h(W2.reshape(W, D, D).transpose(0, 2, 1).reshape(W, D * D))      # [128, 4096]
    Wgh = np.zeros((65, 256), np.float16)
    Wgh[:64, 0:64] = h(conv_root)
    Wgh[:64, 64:256] = h(f(gru_wh).T)
    Wgh[64, 0:64] = h(conv_bias)
    Wgh[64, 64:256] = h(gru_bh)
    Wgi = h(np.concatenate([f(gru_wi).T, f(gru_bi)[None, :]], 0))          # [65, 192]
    Wl1 = h(f(lstm_wi).T[0:64, :] + f(lstm_wh).T)                          # [64, 256]
    Wl2s = h(np.concatenate([f(lstm_wi).T[64:128, :],
                             (f(lstm_bi) + f(lstm_bh))[None, :]], 0))      # [65, 256]
    Wla = h(f(lin1_w)[0:64, :])                                            # [64, 64]
    Wlb = h(np.concatenate([f(lin1_w)[64:128, :], f(lin1_b)[None, :]], 0))  # [65, 64]
    Wf = h(lin2_w)                                                         # [64, 1]
    b2col = np.full((P, 1), float(f(lin2_b).reshape(-1)[0]), np.float32)
    return dict(W0b=W0b, W1b=W1b, W2p=W2p, Wgh=Wgh, Wgi=Wgi, Wl1=Wl1,
                Wl2s=Wl2s, Wla=Wla, Wlb=Wlb, Wf=Wf, b2col=b2col), b2e


# ----------------------------------------------------------------------------
# Device kernel
# ----------------------------------------------------------------------------

def _build(meta):
    NB, SN, GW, ET, SLOTS = meta["NB"], meta["SN"], meta["GW"], meta["ET"], meta["SLOTS"]
    H1B = meta["H1B"]
    Z0 = meta["Z0"]
    AGN = NCORES * SN

    nc = bacc.Bacc("TRN2", target_bir_lowering=False, debug=False,
                   enable_asserts=False, num_devices=NCORES)
    t_xaT = nc.dram_tensor("xaT", [17, SN], F16, kind="ExternalInput")
    t_eaT = nc.dram_tensor("eaT", [6, SLOTS], F16, kind="ExternalInput")
    t_idx = nc.dram_tensor("idx", [P, ET], I32, kind="ExternalInput")
    t_S = nc.dram_tensor("S", [ET, P, P], F16, kind="ExternalInput")
    t_G = nc.dram_tensor("G", [NB * GW, P, P], F16, kind="ExternalInput")
    t_GT = nc.dram_tensor("GT", [NB * GW, P, P], F16, kind="ExternalInput")
    t_ident = nc.dram_tensor("ident", [P, P], F16, kind="ExternalInput")
    t_ident32 = nc.dram_tensor("ident32", [P, P], F32, kind="ExternalInput")
    wt = {}
    for nm, shp, dt in [("W0b", [17, 64], F16), ("W1b", [6, 128], F16),
                        ("W2p", [W, DD], F16), ("Wgh", [65, 256], F16),
                        ("Wgi", [65, 192], F16), ("Wl1", [64, 256], F16),
                        ("Wl2s", [65, 256], F16), ("Wla", [64, 64], F16),
                        ("Wlb", [65, 64], F16), ("Wf", [64, 1], F16),
                        ("b2col", [P, 1], F32)]:
        wt[nm] = nc.dram_tensor(nm, shp, dt, kind="ExternalInput")
    t_y = nc.dram_tensor("y_out", [GW * P], F32, kind="ExternalOutput")

    with nc.allow_low_precision("fp16 message-passing pipeline"), \
         tile.TileContext(nc) as tc:
        with (
            tc.tile_pool(name="persist", bufs=1) as pers,
            tc.tile_pool(name="dram", bufs=1, space="DRAM") as dram,
            tc.tile_pool(name="dram2", bufs=2, space="DRAM") as dram2,
        ):
            idx_sb = pers.tile([P, ET], I32)
            ident = pers.tile([P, P], F16)
            ident32 = pers.tile([P, P], F32)
            nc.sync.dma_start(idx_sb[:], t_idx.ap())
            nc.sync.dma_start(ident[:], t_ident.ap())
            nc.sync.dma_start(ident32[:], t_ident32.ap())

            h_sb = pers.tile([P, NB, D], F16)

            W2p_sb = pers.tile([W, DD], F16)
            Wgh_sb = pers.tile([65, 256], F16)
            Wgi_sb = pers.tile([65, 192], F16)
            nc.sync.dma_start(W2p_sb[:], wt["W2p"].ap())
            nc.sync.dma_start(Wgh_sb[:], wt["Wgh"].ap())
            nc.sync.dma_start(Wgi_sb[:], wt["Wgi"].ap())
            ghT_bufs = [pers.tile([65, P], F16, name=f"ghT{i}", tag=f"ghT{i}")
                        for i in range(3)]
            giT_bufs = [pers.tile([65, P], F16, name=f"giT{i}", tag=f"giT{i}")
                        for i in range(3)]
            for tl in ghT_bufs + giT_bufs:
                nc.gpsimd.memset(tl[64:65, :], 1.0)

            ew_dram = dram.tile([ET, P, DD], F16)


            def ag_alloc():
                hb_dram = dram2.tile([P, NB, D], F16, tag="hbd")
                out_full = dram2.tile([AGN, D], F16, tag="outf",
                                      addr_space="Shared")
                return hb_dram, out_full

            def ag_emit(hb_dram, out_full):
                nc.gpsimd.collective_compute(
                    "AllGather", Alu.bypass,
                    replica_groups=[list(range(NCORES))],
                    ins=[hb_dram.opt()], outs=[out_full.opt()])

            def emit_allgather():
                hb_dram, out_full = ag_alloc()
                nc.scalar.dma_start(hb_dram[:], h_sb[:])
                ag_emit(hb_dram, out_full)
                return out_full

            # ---------------- message passing steps ----------------
            with tc.tile_pool(name="ms", bufs=2) as mssb:
              _hetp_stack = contextlib.ExitStack()
              hetp = _hetp_stack.enter_context(tc.tile_pool(name="het", bufs=1))
              _ph0_stack = contextlib.ExitStack()
              ph0 = _ph0_stack.enter_context(tc.tile_pool(name="ph0", bufs=1))

              _ps = contextlib.ExitStack()
              aggrp = _ps.enter_context(
                  tc.tile_pool(name="aggrp", bufs=2, space="PSUM"))
              ewpsp = _ps.enter_context(
                  tc.tile_pool(name="ewp", bufs=2, space="PSUM"))
              tpp = _ps.enter_context(
                  tc.tile_pool(name="tpp", bufs=1, space="PSUM"))
              gpp = _ps.enter_context(
                  tc.tile_pool(name="gpp", bufs=1, space="PSUM"))
              CH = 8  # GRU interleave chunk (blocks)

              # ---- lin0 + he preamble, fused into the step-0 pipeline ----
              # (shares aggrp/ewpsp PSUM banks; no barrier so the tile
              # scheduler overlaps he production with step 0's block loop)
              xaT_sb = ph0.tile([17, SN], F16)
              W0b_sb = ph0.tile([17, 64], F16)
              W1b_sb = ph0.tile([6, 128], F16)
              nc.gpsimd.dma_start(xaT_sb[:], t_xaT.ap())
              nc.gpsimd.dma_start(W0b_sb[:], wt["W0b"].ap())
              nc.gpsimd.dma_start(W1b_sb[:], wt["W1b"].ap())
              for b in range(NB):
                  h0ps = aggrp.tile([P, D], F32, tag="aggr")
                  nc.tensor.matmul(h0ps[:], xaT_sb[:, b * P:(b + 1) * P],
                                   W0b_sb[:], start=True, stop=True)
                  nc.scalar.activation(h_sb[:, b, :], h0ps[:], Act.Relu)
              out_full = emit_allgather()

              # per-chunk GRU intermediates: small cycling tiles instead of
              # NB-sized arrays (frees ~47KB/partition for deeper ew buffers)
              cur = {}

              def gru_stage_a(b):
                  # gh = [h|1] @ Wgh for block b (uses h from previous step)
                  hT_ps = tpp.tile([D, P], F16, tag="tp")
                  nc.tensor.transpose(hT_ps[:], h_sb[:, b, :], ident[:])
                  ghT = ghT_bufs[b % 3]
                  nc.scalar.copy(ghT[0:64, :], hT_ps[:])
                  gh_ps = gpp.tile([P, 512], F32, tag="ghps")
                  nc.tensor.matmul(gh_ps[:, 0:256], ghT[:], Wgh_sb[:],
                                   start=True, stop=True)
                  nc.scalar.copy(cur["gh"][:, b - cur["c0"], :],
                                 gh_ps[:, 0:256])

              hb_cell = [None]

              def flush_block(b, aggr_ps):
                  if b % CH == 0:
                      cur["c0"] = b
                      cur["gh"] = hetp.tile([P, CH, 256], F16, tag="ghc",
                                            bufs=2, name="ghc")
                      cur["gi"] = hetp.tile([P, CH, 192], F16, tag="gic",
                                            bufs=2, name="gic")
                      cur["ag"] = hetp.tile([P, CH, D], F16, tag="agc",
                                            bufs=2, name="agc")
                      cur["m"] = hetp.tile([P, CH, D], F16, tag="mc",
                                           bufs=2, name="mc")
                      cur["g1"] = hetp.tile([P, CH, D], F16, tag="g1c",
                                            bufs=2, name="g1c")
                      cur["g2"] = hetp.tile([P, CH, D], F16, tag="g2c",
                                            bufs=2, name="g2c")
                  nc.vector.tensor_reduce(
                      cur["ag"][:, b - cur["c0"], :],
                      aggr_ps.rearrange("p (o i) -> p o i", o=D),
                      axis=mybir.AxisListType.X, op=Alu.add)
                  gru_stage_a(b)
                  if b % CH == CH - 1 or b == NB - 1:
                      c0 = b - b % CH
                      gru_stage_b(c0, b + 1)
                      # h for this chunk is final: gate math + stage to DRAM
                      # now so the AllGather launches right after the loop
                      gate_math(c0, b + 1)
                      if hb_cell[0] is not None:
                          nc.scalar.dma_start(hb_cell[0][:, c0:b + 1, :],
                                              h_sb[:, c0:b + 1, :])

              def gate_math(c0, c1):
                  # GRU batched gate math (torch order: r, z, n)
                  n_ = c1 - c0
                  g1h = cur["g1"][:, 0:n_, :]
                  g2h = cur["g2"][:, 0:n_, :]
                  hh_ = h_sb[:, c0:c1, :]
                  gih = cur["gi"][:, 0:n_, :]
                  ghh = cur["gh"][:, 0:n_, :]
                  nc.vector.tensor_tensor(g1h, gih[:, :, 0:64],
                                          ghh[:, :, 64:128], Alu.add)
                  nc.scalar.activation(g1h, g1h, Act.Sigmoid)   # r
                  nc.vector.tensor_tensor(g2h, gih[:, :, 64:128],
                                          ghh[:, :, 128:192], Alu.add)
                  nc.scalar.activation(g2h, g2h, Act.Sigmoid)   # z
                  ngv = gih[:, :, 128:192]
                  nc.vector.tensor_tensor(ghh[:, :, 192:256], g1h,
                                          ghh[:, :, 192:256], Alu.mult)
                  nc.vector.tensor_tensor(ngv, ngv,
                                          ghh[:, :, 192:256], Alu.add)
                  nc.scalar.activation(ngv, ngv, Act.Tanh)          # n
                  nc.vector.tensor_tensor(g1h, hh_, ngv, Alu.subtract)
                  nc.vector.tensor_tensor(g1h, g2h, g1h, Alu.mult)
                  nc.vector.tensor_tensor(hh_, ngv, g1h, Alu.add)

              step_cell = [0]

              def gru_stage_b(c0, c1):
                  # m_pre for chunk, then gi = [relu(m)|1] @ Wgi per block
                  n_ = c1 - c0
                  nc.vector.tensor_tensor(cur["m"][:, 0:n_, :],
                                          cur["ag"][:, 0:n_, :],
                                          cur["gh"][:, 0:n_, 0:64], Alu.add)
                  for b in range(c0, c1):
                      mT_ps = tpp.tile([D, P], F16, tag="tp")
                      nc.tensor.transpose(mT_ps[:], cur["m"][:, b - c0, :],
                                          ident[:])
                      giT = giT_bufs[b % 3]
                      nc.scalar.activation(giT[0:64, :], mT_ps[:], Act.Relu)
                      if step_cell[0] == 0:
                          gi_ps = gpp.tile([P, 512], F32, tag="ghps")
                      else:
                          # ewpsp banks are idle after step 0: 2-deep
                          # pipelining for the gi chain instead of 1
                          gi_ps = ewpsp.tile([P, 512], F32, tag="ewps")
                      nc.tensor.matmul(gi_ps[:, 0:192], giT[:], Wgi_sb[:],
                                       start=True, stop=True)
                      nc.scalar.copy(cur["gi"][:, b - c0, :], gi_ps[:, 0:192])

              for step in range(STEPS):
                step_cell[0] = step
                if step < STEPS - 1:
                    hb_next, of_next = ag_alloc()
                    hb_cell[0] = hb_next
                else:
                    hb_cell[0] = None
                # --- block loop: messages + aggregation, GRU interleaved ---
                if True:
                    pend = None
                    # gathers issued GLA blocks ahead so they clear the DMA
                    # rings before DVE needs them
                    GLA = 3
                    gouts = {}

                    def issue_gather(bb):
                        g = mssb.tile([P, TPB, D], F16, tag="gout", bufs=6,
                                      name="gout")
                        for k in range(TPB):
                            nc.gpsimd.indirect_dma_start(
                                out=g[:, k, :], out_offset=None,
                                in_=out_full[:],
                                in_offset=IndirectOffsetOnAxis(
                                    ap=idx_sb[:, bb * TPB + k:bb * TPB + k + 1],
                                    axis=0))
                        gouts[bb] = g

                    for bb in range(min(GLA, NB)):
                        issue_gather(bb)
                    for b in range(NB):
                        t0 = b * TPB
                        ew_sb = mssb.tile([P, TPB, DD], F16, tag="ewsb",
                                          bufs=3)
                        if step == 0:
                            # he produced just-in-time per tile (eaT streamed)
                            hec = mssb.tile([W, TPB * P], F16, tag="hec",
                                            bufs=3, name="hec")
                            for k in range(TPB):
                                eat = mssb.tile([6, P], F16, tag="eat",
                                                bufs=6, name="eat")
                                nc.sync.dma_start(
                                    eat[:],
                                    t_eaT.ap()[:, (t0 + k) * P:(t0 + k + 1) * P])
                                heps_t = ewpsp.tile([W, P], F32, tag="ewps")
                                nc.tensor.matmul(heps_t[:], W1b_sb[:], eat[:],
                                                 start=True, stop=True)
                                nc.scalar.activation(hec[:, k * P:(k + 1) * P],
                                                     heps_t[:], Act.Relu)
                            for k in range(TPB):
                                for q in range(4):
                                    ewps = ewpsp.tile([P, 1024], F32, tag="ewps")
                                    for hf in range(2):
                                        c0 = q * 1024 + hf * 512
                                        nc.tensor.matmul(
                                            ewps[:, hf * 512:(hf + 1) * 512],
                                            hec[:, k * P:(k + 1) * P],
                                            W2p_sb[:, c0:c0 + 512],
                                            start=True, stop=True)
                                    nc.scalar.copy(
                                        ew_sb[:, k, q * 1024:(q + 1) * 1024],
                                        ewps[:])
                                nc.sync.dma_start(
                                    ew_dram[t0 + k, :, :], ew_sb[:, k, :])
                        else:
                            nc.sync.dma_start(
                                ew_sb[:],
                                ew_dram[t0:t0 + TPB, :, :].rearrange(
                                    "t p d -> p t d"))
                        S_sb = mssb.tile([P, TPB, P], F16, tag="S", bufs=4)
                        nc.sync.dma_start(
                            S_sb[:], t_S.ap()[t0:t0 + TPB].rearrange(
                                "t e v -> e t v"))
                        if b + GLA < NB:
                            issue_gather(b + GLA)
                        gout = gouts.pop(b)
                        # in-place broadcast multiply + halving tree over i
                        ew4 = ew_sb.rearrange("p t (o i) -> p t o i", o=D)
                        msg8 = mssb.tile([P, TPB, D, 8], F16, tag="msg8")
                        if step == 0:
                            # per-tile mults so tile 0's multiply overlaps
                            # tile 1's ew_dram write drain (WAR on ew_sb)
                            for k in range(TPB):
                                nc.vector.tensor_tensor(
                                    ew4[:, k, :, :], ew4[:, k, :, :],
                                    gout[:, k, :].unsqueeze(1)
                                        .broadcast_to([P, D, D]),
                                    Alu.mult)
                        else:
                            nc.vector.tensor_tensor(
                                ew4, ew4,
                                gout.unsqueeze(2).broadcast_to([P, TPB, D, D]),
                                Alu.mult)
                        for half in (32, 16):
                            nc.vector.tensor_tensor(
                                ew4[:, :, :, 0:half], ew4[:, :, :, 0:half],
                                ew4[:, :, :, half:2 * half], Alu.add)
                        nc.vector.tensor_tensor(
                            msg8[:], ew4[:, :, :, 0:8],
                            ew4[:, :, :, 8:16], Alu.add)
                        aggr_ps = aggrp.tile([P, D * 8], F32, tag="aggr")
                        for k in range(TPB):
                            nc.tensor.matmul(
                                aggr_ps[:],
                                S_sb[:, k, :],
                                msg8[:, k, :, :].rearrange("p o i -> p (o i)"),
                                start=(k == 0), stop=(k == TPB - 1))
                        if pend is not None:
                            flush_block(*pend)
                        pend = (b, aggr_ps)
                    flush_block(*pend)
                    pend = None

                if step == 0:
                    _ph0_stack.close()
                    G_sb = mssb.tile([P, NB * GW, P], F16, tag="Gsb",
                                     bufs=1, name="Gsb")
                    for w in range(GW):
                        nc.gpsimd.dma_start(
                            G_sb[:, w:NB * GW:GW, :],
                            t_G.ap()[w:NB * GW:GW].rearrange("t p q -> p t q"))
                if step < STEPS - 1:
                    ag_emit(hb_next, of_next)
                    out_full = of_next

              _ps.close()
              _hetp_stack.close()

              # ---------------- set2set pooling ----------------
              if os.environ.get("K_SKIP_S2S", "0") == "1":
                  with tc.tile_pool(name="ysk", bufs=1) as ysk:
                      y_skip = ysk.tile([P, GW], F32)
                      nc.gpsimd.memset(y_skip[:], 0.0)
                      nc.sync.dma_start(t_y.ap().rearrange("(w p) -> p w", p=P),
                                        y_skip[:])
              elif True:
               with (
                  tc.tile_pool(name="s2s", bufs=1) as s2s,
                  tc.tile_pool(name="s2w", bufs=2) as s2w,
                  tc.tile_pool(name="qgp", bufs=4) as qgp,
                  tc.tile_pool(name="lstp", bufs=1, space="PSUM") as lstp,
                  tc.tile_pool(name="qbp", bufs=2, space="PSUM") as qbp,
                  tc.tile_pool(name="gatp", bufs=2, space="PSUM") as gatp,
                  tc.tile_pool(name="rpp", bufs=1, space="PSUM") as rpp,
              ):
                  hh = s2s.tile([P, GW, D], F32)
                  cc = s2s.tile([P, GW, D], F32)
                  rp = s2s.tile([P, GW, D], F32)
                  hh16 = s2s.tile([P, GW, D], F16)
                  GT_sb = s2s.tile([P, NB * GW, P], F16)
                  wsb = s2s.tile([P, NB, 65], F16)
                  e_sb = s2s.tile([P, NB], F32)
                  ae_sb = s2s.tile([P, NB], F32)
                  l_hh = [s2s.tile([64, P], F16, name=f"lhh{w}", tag=f"lhh{w}")
                          for w in range(GW)]
                  l_rp = [s2s.tile([65, P], F16, name=f"lrp{w}", tag=f"lrp{w}")
                          for w in range(GW)]
                  nc.gpsimd.memset(hh[:], 0.0)
                  nc.gpsimd.memset(cc[:], 0.0)
                  nc.gpsimd.memset(rp[:], 0.0)
                  for w in range(GW):
                      nc.gpsimd.memset(l_rp[w][64:65, :], 1.0)
                  ghalf = (NB * GW + 1) // 2
                  nc.gpsimd.dma_start(
                      GT_sb[:, 0:ghalf, :],
                      t_GT.ap()[0:ghalf].rearrange("t p q -> p t q"))
                  nc.gpsimd.dma_start(
                      GT_sb[:, ghalf:NB * GW, :],
                      t_GT.ap()[ghalf:NB * GW].rearrange("t p q -> p t q"))
                  WL = {}
                  for nm, shp, dt in [("Wl1", [64, 256], F16),
                                      ("Wl2s", [65, 256], F16),
                                      ("Wla", [64, 64], F16),
                                      ("Wlb", [65, 64], F16),
                                      ("Wf", [64, 1], F16),
                                      ("b2col", [P, 1], F32)]:
                      WL[nm] = s2s.tile(shp, dt, tag=nm, name=nm)
                      nc.gpsimd.dma_start(WL[nm][:], wt[nm].ap())

                  def lstm_inputs(w_):
                      hhT_ps = lstp.tile([D, P], F32, tag="ltp")
                      nc.tensor.transpose(hhT_ps[:], hh[:, w_, :], ident32[:])
                      nc.vector.tensor_copy(l_hh[w_][:], hhT_ps[:])
                      rpT_ps = lstp.tile([D, P], F32, tag="ltp")
                      nc.tensor.transpose(rpT_ps[:], rp[:, w_, :], ident32[:])
                      nc.vector.tensor_copy(l_rp[w_][0:64, :], rpT_ps[:])

                  for it in range(STEPS):
                      if it == 0 and Z0:
                          # zero LSTM state + zero biases => q == 0 exactly,
                          # so softmax weights are uniform: wsb = [h | 1]
                          nc.vector.tensor_copy(wsb[:, :, 0:64], h_sb[:])
                          nc.gpsimd.memset(wsb[:, :, 64:65], 1.0)
                      else:
                        # --- LSTM update per graph window ---
                        for w_ in range(GW):
                          lstm_inputs(w_)
                          g_ps = gatp.tile([P, 256], F32, tag="gat")
                          nc.tensor.matmul(g_ps[:], l_hh[w_][:], WL["Wl1"][:],
                                           start=True, stop=False)
                          nc.tensor.matmul(g_ps[:], l_rp[w_][:], WL["Wl2s"][:],
                                           start=False, stop=True)
                          ig = s2w.tile([P, D], F16, tag="ig")
                          nc.scalar.activation(ig[:], g_ps[:, 0:64], Act.Sigmoid)
                          fg = s2w.tile([P, D], F16, tag="fg")
                          nc.scalar.activation(fg[:], g_ps[:, 64:128], Act.Sigmoid)
                          gg = s2w.tile([P, D], F16, tag="gg")
                          nc.scalar.activation(gg[:], g_ps[:, 128:192], Act.Tanh)
                          og = s2w.tile([P, D], F16, tag="og")
                          nc.scalar.activation(og[:], g_ps[:, 192:256], Act.Sigmoid)
                          t1 = s2w.tile([P, D], F32, tag="t1")
                          nc.vector.tensor_mul(t1[:], fg[:], cc[:, w_, :])
                          t2 = s2w.tile([P, D], F16, tag="t2")
                          nc.vector.tensor_mul(t2[:], ig[:], gg[:])
                          nc.vector.tensor_add(cc[:, w_, :], t1[:], t2[:])
                          tc_ = s2w.tile([P, D], F16, tag="tc")
                          nc.scalar.activation(tc_[:], cc[:, w_, :], Act.Tanh)
                          nc.vector.tensor_mul(hh[:, w_, :], og[:], tc_[:])
                          nc.vector.tensor_copy(hh16[:, w_, :], hh[:, w_, :])
                        # --- q[batch] via GT matmuls (PE), then batched e ---
                        qg = qgp.tile([P, NB, D], F16, tag="qg", bufs=2)
                        for b0 in range(0, NB, 8):
                          b1 = min(b0 + 8, NB)
                          qb_ps = qbp.tile([P, 8, D], F32, tag="qb")
                          for b in range(b0, b1):
                              for w_ in range(GW):
                                  nc.tensor.matmul(qb_ps[:, b - b0, :],
                                                   GT_sb[:, b * GW + w_, :],
                                                   hh16[:, w_, :],
                                                   start=(w_ == 0),
                                                   stop=(w_ == GW - 1))
                          nc.scalar.copy(qg[:, b0:b1, :], qb_ps[:, 0:b1 - b0, :])
                        ep_ = qgp.tile([P, NB, D], F16, tag="ep", bufs=2)
                        nc.vector.tensor_tensor(ep_[:], h_sb[:], qg[:], Alu.mult)
                        nc.vector.tensor_reduce(
                            e_sb[:], ep_[:],
                            axis=mybir.AxisListType.X, op=Alu.add)
                        nc.scalar.activation(ae_sb[:], e_sb[:], Act.Exp)
                        nc.vector.tensor_tensor(
                            wsb[:, :, 0:64], h_sb[:],
                            ae_sb.unsqueeze(2).broadcast_to([P, NB, D]),
                            Alu.mult)
                        nc.vector.tensor_copy(wsb[:, :, 64:65],
                                              ae_sb.unsqueeze(2))
                      # --- r_pool + asum via G matmuls ---
                      for w_ in range(GW):
                          rp_ps = rpp.tile([P, 65], F32, tag="rp")
                          for b in range(NB):
                              nc.tensor.matmul(rp_ps[:], G_sb[:, b * GW + w_, :],
                                               wsb[:, b, :],
                                               start=(b == 0), stop=(b == NB - 1))
                          asum = s2w.tile([P, 1], F32, tag="asum")
                          nc.vector.tensor_scalar_add(asum[:], rp_ps[:, 64:65],
                                                      1e-16)
                          rec = s2w.tile([P, 1], F32, tag="rec")
                          nc.vector.reciprocal(rec[:], asum[:])
                          nc.vector.tensor_scalar_mul(rp[:, w_, :],
                                                      rp_ps[:, 0:64], rec[:])

                  # --- final readout ---
                  y_sb = s2s.tile([P, GW], F32)
                  for w_ in range(GW):
                      lstm_inputs(w_)
                      t_ps = gatp.tile([P, 256], F32, tag="gat")
                      nc.tensor.matmul(t_ps[:, 0:64], l_hh[w_][:], WL["Wla"][:],
                                       start=True, stop=False)
                      nc.tensor.matmul(t_ps[:, 0:64], l_rp[w_][:], WL["Wlb"][:],
                                       start=False, stop=True)
                      t_sb = s2w.tile([P, D], F16, tag="tsb")
                      nc.scalar.activation(t_sb[:], t_ps[:, 0:64], Act.Relu)
                      tT_ps = lstp.tile([D, P], F16, tag="ttp")
                      nc.tensor.transpose(tT_ps[:], t_sb[:], ident[:])
                      tT_sb = s2w.tile([64, P], F16, tag="ttsb")
                      nc.vector.tensor_copy(tT_sb[:], tT_ps[:])
                      y_ps = rpp.tile([P, 1], F32, tag="yp")
                      nc.tensor.matmul(y_ps[:], tT_sb[:], WL["Wf"][:],
                                       start=True, stop=True)
                      nc.vector.tensor_scalar_add(y_sb[:, w_:w_ + 1], y_ps[:],
                                                  WL["b2col"][:])
                  nc.sync.dma_start(t_y.ap().rearrange("(w p) -> p w", p=P),
                                    y_sb[:])

    nc.compile()
    return nc


# ----------------------------------------------------------------------------
# Entry point
# ----------------------------------------------------------------------------

def kernel(**inputs):
    x = np.asarray(inputs["x"], np.float32)
    z = np.asarray(inputs["z"], np.float32)
    edge_attr = np.asarray(inputs["edge_attr"], np.float32)
    edge_index = np.asarray(inputs["edge_index"]).astype(np.int64)
    batch = np.asarray(inputs["batch"]).astype(np.int64)
    num_graphs = int(np.asarray(inputs["num_graphs"]))

    wts, b2e = _weights(*[inputs[k] for k in
                          ["lin0_w", "lin0_b", "emlp_w1", "emlp_b1", "emlp_w2",
                           "emlp_b2", "conv_root", "conv_bias", "gru_wi",
                           "gru_wh", "gru_bi", "gru_bh", "lstm_wi", "lstm_wh",
                           "lstm_bi", "lstm_bh", "lin1_w", "lin1_b", "lin2_w",
                           "lin2_b"]])
    assert np.all(b2e == 0.0), "nonzero emlp_b2 not supported"

    per_rank, meta = _preprocess(x, z, edge_attr, edge_index, batch, num_graphs)
    meta["Z0"] = bool(
        np.all(np.asarray(inputs["lstm_bi"], np.float32) == 0.0)
        and np.all(np.asarray(inputs["lstm_bh"], np.float32) == 0.0))
    nc = _build(meta)

    ident = np.eye(P, dtype=np.float16)
    ident32 = np.eye(P, dtype=np.float32)
    in_maps = []
    for r in range(NCORES):
        pr = per_rank[r]
        m = dict(xaT=pr["xaT"], eaT=pr["eaT"], idx=pr["idx"], S=pr["S"],
                 G=pr["G"], GT=pr["GT"], ident=ident, ident32=ident32,
                 **wts)
        in_maps.append(m)

    res = run_bass_kernel_spmd(nc, in_maps, core_ids=list(range(NCORES)))
    if res.exec_time_ns is not None:
        print(f"HW exec time: {res.exec_time_ns} ns")

    ys = []
    for r in range(NCORES):
        ys.append(res.results[r]["y_out"][:per_rank[r]["ngr"]])
    return np.concatenate(ys).astype(np.float32)



# revision 46
# speedup vs baseline: 1.0107x; 1.0107x over previous
"""Trainium2 Bass kernel for nn_Net_NNCONV (gnn_message_passing).

Strategy (8-core SPMD, data-parallel by graph):
 - Host: cut graphs into 8 contiguous shards (balanced node counts); within a
   shard, bin-pack nodes into NB 128-node blocks with <=256 in-edges per block
   (TPB=2 edge tiles per block); edges grouped by dst block in slot order.
 - fp16 everywhere on device (PE fp16 = 1 cycle/row vs 4 for f32; DVE 2x mode
   needs 2-byte packed operands).
 - Phase 0: lin0 -> h0 fp16; AllGather h0; he = relu(ea@W1+b1) fp16 in SBUF.
 - Step 0 fuses the ew = he@W2 production (PE matmuls + ACT psum->fp16 casts)
   into the message block loop: the fp16 ew tile is consumed directly and also
   written to DRAM for steps 1-4, which stream it back.
 - Per block (256 edges): indirect-gather gout fp16, in-place broadcast
   multiply (DVE 2x), in-place halving-tree partial reduce 64->8 over i,
   S-matmul (fp16 one-hot, 1/deg folded) with rhs [64o,8i] into PSUM
   accumulating the block's 2 tiles, single 1x reduce psum->aggr fp16.
 - GRU: per-block PE transposes + gh/gi matmuls (fp16), ACT psum copies,
   gate math batched over all blocks as [128, NB*64] fp16 DVE ops.
 - Set2Set: per-node q via fp16 GT one-hot matmuls on the (idle) PE; e =
   rowsum(h * q) as batched DVE mult+reduce (tensor_tensor_reduce crashes the
   device); r_pool/asum via one-hot G matmuls (G fp16, SBUF-resident); softmax
   skips max-subtraction (args O(10), fp32 exp).
 - ew_sb is triple-buffered so the 2 MB/block fabric-limited DMA is fully
   prefetched; the block psum reduce + GRU stages are software-pipelined one
   block behind the S-matmuls so DVE never waits on PE.
 - lin0/he preamble is fused into the step-0 pipeline (shared PSUM pools, no
   barrier); GRU gate math for the first block-half is folded into the block
   loop so only half remains serial before each AllGather.
 - Set2Set: iteration 0 is algebraic (zero LSTM biases => q==0 => uniform
   softmax), so it reduces to a mean-pool; q-copies batched 8 blocks/copy;
   G loaded w-major on the ACT queue so iter-0 r_pool starts early.
 - Measured bounds (trn2): DVE fp16 tensor_tensor caps at 2 elem/cyc/lane
   (mult 4.3us + tree 3.7us per 256-edge block ~= 440us/step floor); Pool
   tensor_tensor is ~7.6x slower AND contends with DVE for SBUF ports (never
   offload elementwise there); multi-column indirect_dma_start crashes the
   device; Shared collective outputs allow only one writer instruction.
"""
import contextlib
import os
import sys

sys.path.insert(0, "/opt/trn_rl_repo")

import numpy as np

import concourse.bass as bass
import concourse.tile as tile
from concourse import bacc, mybir
from concourse.bass import IndirectOffsetOnAxis
from concourse.bass_utils import run_bass_kernel_spmd

F32 = mybir.dt.float32
F16 = mybir.dt.float16
I32 = mybir.dt.int32
NCORES = 8
P = 128
D = 64
DD = D * D
W = 128
TPB = 2
EPB = TPB * P          # max edges per block
STEPS = 5
Alu = mybir.AluOpType
Act = mybir.ActivationFunctionType


# ----------------------------------------------------------------------------
# Host-side preprocessing
# ----------------------------------------------------------------------------

def _pack_rank(deg, lo, hi, NB):
    """Assign nodes [lo,hi) to NB blocks: <=128 nodes, <=EPB in-edges each.
    Returns (block, pos) per node or None if packing fails."""
    nodes = np.arange(lo, hi)
    order = nodes[np.argsort(-deg[nodes], kind="stable")]
    loads = np.zeros(NB, dtype=np.int64)
    counts = np.zeros(NB, dtype=np.int64)
    blk = np.zeros(hi - lo, dtype=np.int64)
    for nd in order:
        d = deg[nd]
        cand = np.flatnonzero(counts < P)
        if len(cand) == 0:
            return None
        bi = cand[np.argmin(loads[cand])]
        if loads[bi] + d > EPB:
            return None
        blk[nd - lo] = bi
        loads[bi] += d
        counts[bi] += 1
    pos = np.zeros(hi - lo, dtype=np.int64)
    ctr = np.zeros(NB, dtype=np.int64)
    for nd in order:
        b = blk[nd - lo]
        pos[nd - lo] = ctr[b]
        ctr[b] += 1
    return blk, pos


def _preprocess(x, z, edge_attr, edge_index, batch, num_graphs):
    N = x.shape[0]
    E = edge_index.shape[1]
    B = int(num_graphs)
    src = np.asarray(edge_index[0], dtype=np.int64)
    dst = np.asarray(edge_index[1], dtype=np.int64)
    batch = np.asarray(batch, dtype=np.int64)

    # --- shard cut: contiguous graphs, balanced node counts ---
    gcounts = np.bincount(batch, minlength=B)
    gcum = np.concatenate([[0], np.cumsum(gcounts)])
    targets = (np.arange(1, NCORES) * N) / NCORES
    cuts = np.searchsorted(gcum, targets)
    gcut = np.concatenate([[0], cuts, [B]])
    n0 = gcum[gcut]
    nodes_per = np.diff(n0)
    graphs_per = np.diff(gcut)

    GW = int(np.ceil(graphs_per.max() / P))

    deg = np.bincount(dst, minlength=N).astype(np.int64)
    dinv = (1.0 / np.maximum(deg, 1)).astype(np.float32)
    e_rank = np.searchsorted(n0[1:], dst, side="right")
    edges_per = np.bincount(e_rank, minlength=NCORES)

    NB = int(max(np.ceil(nodes_per.max() / P), np.ceil(edges_per.max() / EPB)))
    packs = None
    while packs is None:
        packs = []
        for r in range(NCORES):
            res = _pack_rank(deg, int(n0[r]), int(n0[r + 1]), NB)
            if res is None:
                packs = None
                NB += 1
                break
            packs.append(res)

    ET = NB * TPB
    SN = NB * P
    SLOTS = ET * P

    # AllGather is emitted in two block-halves; gathered rows live at
    # [rank-major half-1 | rank-major half-2] in out_full.
    H1B = NB // 2
    H1 = H1B * P
    H2 = (NB - H1B) * P

    node_block = np.zeros(N, dtype=np.int64)
    node_pos = np.zeros(N, dtype=np.int64)
    pad_global = np.zeros(N, dtype=np.int64)
    ag_global = np.zeros(N, dtype=np.int64)
    for r in range(NCORES):
        lo, hi = int(n0[r]), int(n0[r + 1])
        blk, pos = packs[r]
        node_block[lo:hi] = blk
        node_pos[lo:hi] = pos
        pad_global[lo:hi] = r * SN + blk * P + pos
        ag_global[lo:hi] = r * SN + pos * NB + blk

    xa = np.concatenate([np.asarray(x, np.float32),
                         np.asarray(z, np.float32)[:, None],
                         np.ones((N, 1), np.float32)], axis=1)  # [N, 17]

    per_rank = []
    for r in range(NCORES):
        lo, hi = int(n0[r]), int(n0[r + 1])
        g0 = int(gcut[r])
        ngr = int(gcut[r + 1] - gcut[r])

        xaT = np.zeros((17, SN), np.float16)
        loc = pad_global[lo:hi] - r * SN
        xaT[:, loc] = xa[lo:hi].T.astype(np.float16)

        em = e_rank == r
        es, ed = src[em], dst[em]
        eb = node_block[ed]
        eorder = np.argsort(eb, kind="stable")
        es, ed, eb = es[eorder], ed[eorder], eb[eorder]
        ea_r = np.asarray(edge_attr, np.float32)[em][eorder]

        eaT = np.zeros((6, SLOTS), np.float16)
        src_idx = np.zeros(SLOTS, np.int32)
        S = np.zeros((ET, P, P), np.float16)
        bstart = np.searchsorted(eb, np.arange(NB))
        bend = np.searchsorted(eb, np.arange(NB), side="right")
        for b in range(NB):
            cnt = bend[b] - bstart[b]
            assert cnt <= EPB, f"rank {r} block {b} edges {cnt} > {EPB}"
            sl = slice(bstart[b], bend[b])
            base = b * EPB
            eaT[:5, base:base + cnt] = ea_r[sl].T.astype(np.float16)
            eaT[5, base:base + cnt] = 1.0
            src_idx[base:base + cnt] = ag_global[es[sl]].astype(np.int32)
            vcol = node_pos[ed[sl]].astype(np.int64)
            vals = dinv[ed[sl]]
            within = np.arange(cnt)
            S[b * TPB + within // P, within % P, vcol] = vals.astype(np.float16)

        idx2d = src_idx.reshape(ET, P).T.copy()  # [128, ET]

        # pooling one-hot G[(b,w)][node_p, g_col] (+ transpose for q lookup)
        G = np.zeros((NB * GW, P, P), np.float16)
        gl = batch[lo:hi] - g0
        bb = node_block[lo:hi]
        pp = node_pos[lo:hi]
        ww = gl // P
        cc_ = gl % P
        G[bb * GW + ww, pp, cc_] = 1.0
        GT = np.ascontiguousarray(np.transpose(G, (0, 2, 1)))

        per_rank.append(dict(xaT=xaT, eaT=eaT, idx=idx2d, S=S, G=G, GT=GT,
                             ngr=ngr))

    meta = dict(NB=NB, SN=SN, GW=GW, ET=ET, SLOTS=SLOTS, H1B=H1B)
    return per_rank, meta


def _weights(lin0_w, lin0_b, emlp_w1, emlp_b1, emlp_w2, emlp_b2,
             conv_root, conv_bias, gru_wi, gru_wh, gru_bi, gru_bh,
             lstm_wi, lstm_wh, lstm_bi, lstm_bh, lin1_w, lin1_b, lin2_w, lin2_b):
    f = lambda a: np.asarray(a, np.float32)
    h = lambda a: np.asarray(a, np.float16)
    W0b = h(np.concatenate([f(lin0_w), f(lin0_b)[None, :]], 0))            # [17, 64]
    W1b = h(np.concatenate([f(emlp_w1), f(emlp_b1)[None, :]], 0))          # [6, 128]
    W2 = f(emlp_w2)
    b2e = f(emlp_b2)
    W2p = h(W2.reshape(W, D, D).transpose(0, 2, 1).reshape(W, D * D))      # [128, 4096]
    Wgh = np.zeros((65, 256), np.float16)
    Wgh[:64, 0:64] = h(conv_root)
    Wgh[:64, 64:256] = h(f(gru_wh).T)
    Wgh[64, 0:64] = h(conv_bias)
    Wgh[64, 64:256] = h(gru_bh)
    Wgi = h(np.concatenate([f(gru_wi).T, f(gru_bi)[None, :]], 0))          # [65, 192]
    Wl1 = h(f(lstm_wi).T[0:64, :] + f(lstm_wh).T)                          # [64, 256]
    Wl2s = h(np.concatenate([f(lstm_wi).T[64:128, :],
                             (f(lstm_bi) + f(lstm_bh))[None, :]], 0))      # [65, 256]
    Wla = h(f(lin1_w)[0:64, :])                                            # [64, 64]
    Wlb = h(np.concatenate([f(lin1_w)[64:128, :], f(lin1_b)[None, :]], 0))  # [65, 64]
    Wf = h(lin2_w)                                                         # [64, 1]
    b2col = np.full((P, 1), float(f(lin2_b).reshape(-1)[0]), np.float32)
    return dict(W0b=W0b, W1b=W1b, W2p=W2p, Wgh=Wgh, Wgi=Wgi, Wl1=Wl1,
                Wl2s=Wl2s, Wla=Wla, Wlb=Wlb, Wf=Wf, b2col=b2col), b2e


# ----------------------------------------------------------------------------
# Device kernel
# ----------------------------------------------------------------------------

def _build(meta):
    NB, SN, GW, ET, SLOTS = meta["NB"], meta["SN"], meta["GW"], meta["ET"], meta["SLOTS"]
    H1B = meta["H1B"]
    Z0 = meta["Z0"]
    AGN = NCORES * SN

    nc = bacc.Bacc("TRN2", target_bir_lowering=False, debug=False,
                   enable_asserts=False, num_devices=NCORES)
    t_xaT = nc.dram_tensor("xaT", [17, SN], F16, kind="ExternalInput")
    t_eaT = nc.dram_tensor("eaT", [6, SLOTS], F16, kind="ExternalInput")
    t_idx = nc.dram_tensor("idx", [P, ET], I32, kind="ExternalInput")
    t_S = nc.dram_tensor("S", [ET, P, P], F16, kind="ExternalInput")
    t_G = nc.dram_tensor("G", [NB * GW, P, P], F16, kind="ExternalInput")
    t_GT = nc.dram_tensor("GT", [NB * GW, P, P], F16, kind="ExternalInput")
    t_ident = nc.dram_tensor("ident", [P, P], F16, kind="ExternalInput")
    t_ident32 = nc.dram_tensor("ident32", [P, P], F32, kind="ExternalInput")
    wt = {}
    for nm, shp, dt in [("W0b", [17, 64], F16), ("W1b", [6, 128], F16),
                        ("W2p", [W, DD], F16), ("Wgh", [65, 256], F16),
                        ("Wgi", [65, 192], F16), ("Wl1", [64, 256], F16),
                        ("Wl2s", [65, 256], F16), ("Wla", [64, 64], F16),
                        ("Wlb", [65, 64], F16), ("Wf", [64, 1], F16),
                        ("b2col", [P, 1], F32)]:
        wt[nm] = nc.dram_tensor(nm, shp, dt, kind="ExternalInput")
    t_y = nc.dram_tensor("y_out", [GW * P], F32, kind="ExternalOutput")

    with nc.allow_low_precision("fp16 message-passing pipeline"), \
         tile.TileContext(nc) as tc:
        with (
            tc.tile_pool(name="persist", bufs=1) as pers,
            tc.tile_pool(name="dram", bufs=1, space="DRAM") as dram,
            tc.tile_pool(name="dram2", bufs=2, space="DRAM") as dram2,
        ):
            idx_sb = pers.tile([P, ET], I32)
            ident = pers.tile([P, P], F16)
            ident32 = pers.tile([P, P], F32)
            nc.sync.dma_start(idx_sb[:], t_idx.ap())
            nc.sync.dma_start(ident[:], t_ident.ap())
            nc.sync.dma_start(ident32[:], t_ident32.ap())

            h_sb = pers.tile([P, NB, D], F16)

            W2p_sb = pers.tile([W, DD], F16)
            Wgh_sb = pers.tile([65, 256], F16)
            Wgi_sb = pers.tile([65, 192], F16)
            nc.sync.dma_start(W2p_sb[:], wt["W2p"].ap())
            nc.sync.dma_start(Wgh_sb[:], wt["Wgh"].ap())
            nc.sync.dma_start(Wgi_sb[:], wt["Wgi"].ap())
            ghT_bufs = [pers.tile([65, P], F16, name=f"ghT{i}", tag=f"ghT{i}")
                        for i in range(3)]
            giT_bufs = [pers.tile([65, P], F16, name=f"giT{i}", tag=f"giT{i}")
                        for i in range(3)]
            for tl in ghT_bufs + giT_bufs:
                nc.gpsimd.memset(tl[64:65, :], 1.0)

            ew_dram = dram.tile([ET, P, DD], F16)


            def ag_alloc():
                hb_dram = dram2.tile([P, NB, D], F16, tag="hbd")
                out_full = dram2.tile([AGN, D], F16, tag="outf",
                                      addr_space="Shared")
                return hb_dram, out_full

            def ag_emit(hb_dram, out_full):
                nc.gpsimd.collective_compute(
                    "AllGather", Alu.bypass,
                    replica_groups=[list(range(NCORES))],
                    ins=[hb_dram.opt()], outs=[out_full.opt()])

            def emit_allgather():
                hb_dram, out_full = ag_alloc()
                nc.scalar.dma_start(hb_dram[:], h_sb[:])
                ag_emit(hb_dram, out_full)
                return out_full

            # ---------------- message passing steps ----------------
            with tc.tile_pool(name="ms", bufs=2) as mssb:
              _hetp_stack = contextlib.ExitStack()
              hetp = _hetp_stack.enter_context(tc.tile_pool(name="het", bufs=1))
              _ph0_stack = contextlib.ExitStack()
              ph0 = _ph0_stack.enter_context(tc.tile_pool(name="ph0", bufs=1))
              heT = ph0.tile([W, SLOTS], F16)
              _ps = contextlib.ExitStack()
              aggrp = _ps.enter_context(
                  tc.tile_pool(name="aggrp", bufs=2, space="PSUM"))
              ewpsp = _ps.enter_context(
                  tc.tile_pool(name="ewp", bufs=2, space="PSUM"))
              tpp = _ps.enter_context(
                  tc.tile_pool(name="tpp", bufs=1, space="PSUM"))
              gpp = _ps.enter_context(
                  tc.tile_pool(name="gpp", bufs=1, space="PSUM"))
              CH = 8  # GRU interleave chunk (blocks)

              # ---- lin0 + he preamble, fused into the step-0 pipeline ----
              # (shares aggrp/ewpsp PSUM banks; no barrier so the tile
              # scheduler overlaps he production with step 0's block loop)
              xaT_sb = ph0.tile([17, SN], F16)
              W0b_sb = ph0.tile([17, 64], F16)
              eaT_sb = ph0.tile([6, SLOTS], F16)
              W1b_sb = ph0.tile([6, 128], F16)
              nc.gpsimd.dma_start(xaT_sb[:], t_xaT.ap())
              nc.gpsimd.dma_start(W0b_sb[:], wt["W0b"].ap())
              nc.gpsimd.dma_start(eaT_sb[:], t_eaT.ap())
              nc.gpsimd.dma_start(W1b_sb[:], wt["W1b"].ap())
              for b in range(NB):
                  h0ps = aggrp.tile([P, D], F32, tag="aggr")
                  nc.tensor.matmul(h0ps[:], xaT_sb[:, b * P:(b + 1) * P],
                                   W0b_sb[:], start=True, stop=True)
                  nc.scalar.activation(h_sb[:, b, :], h0ps[:], Act.Relu)
              out_full = emit_allgather()
              for t in range(ET):
                  heps_t = ewpsp.tile([W, P], F32, tag="ewps")
                  nc.tensor.matmul(heps_t[:], W1b_sb[:],
                                   eaT_sb[:, t * P:(t + 1) * P],
                                   start=True, stop=True)
                  nc.scalar.activation(heT[:, t * P:(t + 1) * P],
                                       heps_t[:], Act.Relu)

              # per-chunk GRU intermediates: small cycling tiles instead of
              # NB-sized arrays (frees ~47KB/partition for deeper ew buffers)
              cur = {}

              def gru_stage_a(b):
                  # gh = [h|1] @ Wgh for block b (uses h from previous step)
                  hT_ps = tpp.tile([D, P], F16, tag="tp")
                  nc.tensor.transpose(hT_ps[:], h_sb[:, b, :], ident[:])
                  ghT = ghT_bufs[b % 3]
                  nc.scalar.copy(ghT[0:64, :], hT_ps[:])
                  gh_ps = gpp.tile([P, 512], F32, tag="ghps")
                  nc.tensor.matmul(gh_ps[:, 0:256], ghT[:], Wgh_sb[:],
                                   start=True, stop=True)
                  nc.scalar.copy(cur["gh"][:, b - cur["c0"], :],
                                 gh_ps[:, 0:256])

              hb_cell = [None]

              def flush_block(b, aggr_ps):
                  if b % CH == 0:
                      cur["c0"] = b
                      cur["gh"] = hetp.tile([P, CH, 256], F16, tag="ghc",
                                            bufs=2, name="ghc")
                      cur["gi"] = hetp.tile([P, CH, 192], F16, tag="gic",
                                            bufs=2, name="gic")
                      cur["ag"] = hetp.tile([P, CH, D], F16, tag="agc",
                                            bufs=2, name="agc")
                      cur["m"] = hetp.tile([P, CH, D], F16, tag="mc",
                                           bufs=2, name="mc")
                      cur["g1"] = hetp.tile([P, CH, D], F16, tag="g1c",
                                            bufs=2, name="g1c")
                      cur["g2"] = hetp.tile([P, CH, D], F16, tag="g2c",
                                            bufs=2, name="g2c")
                  nc.vector.tensor_reduce(
                      cur["ag"][:, b - cur["c0"], :],
                      aggr_ps.rearrange("p (o i) -> p o i", o=D),
                      axis=mybir.AxisListType.X, op=Alu.add)
                  gru_stage_a(b)
                  if b % CH == CH - 1 or b == NB - 1:
                      c0 = b - b % CH
                      gru_stage_b(c0, b + 1)
                      # h for this chunk is final: gate math + stage to DRAM
                      # now so the AllGather launches right after the loop
                      gate_math(c0, b + 1)
                      if hb_cell[0] is not None:
                          nc.scalar.dma_start(hb_cell[0][:, c0:b + 1, :],
                                              h_sb[:, c0:b + 1, :])

              def gate_math(c0, c1):
                  # GRU batched gate math (torch order: r, z, n)
                  n_ = c1 - c0
                  g1h = cur["g1"][:, 0:n_, :]
                  g2h = cur["g2"][:, 0:n_, :]
                  hh_ = h_sb[:, c0:c1, :]
                  gih = cur["gi"][:, 0:n_, :]
                  ghh = cur["gh"][:, 0:n_, :]
                  nc.vector.tensor_tensor(g1h, gih[:, :, 0:64],
                                          ghh[:, :, 64:128], Alu.add)
                  nc.scalar.activation(g1h, g1h, Act.Sigmoid)   # r
                  nc.vector.tensor_tensor(g2h, gih[:, :, 64:128],
                                          ghh[:, :, 128:192], Alu.add)
                  nc.scalar.activation(g2h, g2h, Act.Sigmoid)   # z
                  ngv = gih[:, :, 128:192]
                  nc.vector.tensor_tensor(ghh[:, :, 192:256], g1h,
                                          ghh[:, :, 192:256], Alu.mult)
                  nc.vector.tensor_tensor(ngv, ngv,
                                          ghh[:, :, 192:256], Alu.add)
                  nc.scalar.activation(ngv, ngv, Act.Tanh)          # n
                  nc.vector.tensor_tensor(g1h, hh_, ngv, Alu.subtract)
                  nc.vector.tensor_tensor(g1h, g2h, g1h, Alu.mult)
                  nc.vector.tensor_tensor(hh_, ngv, g1h, Alu.add)

              step_cell = [0]

              def gru_stage_b(c0, c1):
                  # m_pre for chunk, then gi = [relu(m)|1] @ Wgi per block
                  n_ = c1 - c0
                  nc.vector.tensor_tensor(cur["m"][:, 0:n_, :],
                                          cur["ag"][:, 0:n_, :],
                                          cur["gh"][:, 0:n_, 0:64], Alu.add)
                  for b in range(c0, c1):
                      mT_ps = tpp.tile([D, P], F16, tag="tp")
                      nc.tensor.transpose(mT_ps[:], cur["m"][:, b - c0, :],
                                          ident[:])
                      giT = giT_bufs[b % 3]
                      nc.scalar.activation(giT[0:64, :], mT_ps[:], Act.Relu)
                      if step_cell[0] == 0:
                          gi_ps = gpp.tile([P, 512], F32, tag="ghps")
                      else:
                          # ewpsp banks are idle after step 0: 2-deep
                          # pipelining for the gi chain instead of 1
                          gi_ps = ewpsp.tile([P, 512], F32, tag="ewps")
                      nc.tensor.matmul(gi_ps[:, 0:192], giT[:], Wgi_sb[:],
                                       start=True, stop=True)
                      nc.scalar.copy(cur["gi"][:, b - c0, :], gi_ps[:, 0:192])

              for step in range(STEPS):
                step_cell[0] = step
                if step < STEPS - 1:
                    hb_next, of_next = ag_alloc()
                    hb_cell[0] = hb_next
                else:
                    hb_cell[0] = None
                # --- block loop: messages + aggregation, GRU interleaved ---
                if True:
                    pend = None
                    # gathers issued GLA blocks ahead so they clear the DMA
                    # rings before DVE needs them
                    GLA = 3
                    gouts = {}

                    def issue_gather(bb):
                        g = mssb.tile([P, TPB, D], F16, tag="gout", bufs=6,
                                      name="gout")
                        for k in range(TPB):
                            nc.gpsimd.indirect_dma_start(
                                out=g[:, k, :], out_offset=None,
                                in_=out_full[:],
                                in_offset=IndirectOffsetOnAxis(
                                    ap=idx_sb[:, bb * TPB + k:bb * TPB + k + 1],
                                    axis=0))
                        gouts[bb] = g

                    for bb in range(min(GLA, NB)):
                        issue_gather(bb)
                    for b in range(NB):
                        t0 = b * TPB
                        ew_sb = mssb.tile([P, TPB, DD], F16, tag="ewsb",
                                          bufs=3)
                        if step == 0:
                            for k in range(TPB):
                                for q in range(4):
                                    ewps = ewpsp.tile([P, 1024], F32, tag="ewps")
                                    for hf in range(2):
                                        c0 = q * 1024 + hf * 512
                                        nc.tensor.matmul(
                                            ewps[:, hf * 512:(hf + 1) * 512],
                                            heT[:, (t0 + k) * P:(t0 + k + 1) * P],
                                            W2p_sb[:, c0:c0 + 512],
                                            start=True, stop=True)
                                    nc.scalar.copy(
                                        ew_sb[:, k, q * 1024:(q + 1) * 1024],
                                        ewps[:])
                                nc.sync.dma_start(
                                    ew_dram[t0 + k, :, :], ew_sb[:, k, :])
                        else:
                            nc.sync.dma_start(
                                ew_sb[:],
                                ew_dram[t0:t0 + TPB, :, :].rearrange(
                                    "t p d -> p t d"))
                        S_sb = mssb.tile([P, TPB, P], F16, tag="S", bufs=4)
                        nc.sync.dma_start(
                            S_sb[:], t_S.ap()[t0:t0 + TPB].rearrange(
                                "t e v -> e t v"))
                        if b + GLA < NB:
                            issue_gather(b + GLA)
                        gout = gouts.pop(b)
                        # in-place broadcast multiply + halving tree over i
                        ew4 = ew_sb.rearrange("p t (o i) -> p t o i", o=D)
                        msg8 = mssb.tile([P, TPB, D, 8], F16, tag="msg8")
                        if step == 0:
                            # per-tile mults so tile 0's multiply overlaps
                            # tile 1's ew_dram write drain (WAR on ew_sb)
                            for k in range(TPB):
                                nc.vector.tensor_tensor(
                                    ew4[:, k, :, :], ew4[:, k, :, :],
                                    gout[:, k, :].unsqueeze(1)
                                        .broadcast_to([P, D, D]),
                                    Alu.mult)
                        else:
                            nc.vector.tensor_tensor(
                                ew4, ew4,
                                gout.unsqueeze(2).broadcast_to([P, TPB, D, D]),
                                Alu.mult)
                        for half in (32, 16):
                            nc.vector.tensor_tensor(
                                ew4[:, :, :, 0:half], ew4[:, :, :, 0:half],
                                ew4[:, :, :, half:2 * half], Alu.add)
                        nc.vector.tensor_tensor(
                            msg8[:], ew4[:, :, :, 0:8],
                            ew4[:, :, :, 8:16], Alu.add)
                        aggr_ps = aggrp.tile([P, D * 8], F32, tag="aggr")
                        for k in range(TPB):
                            nc.tensor.matmul(
                                aggr_ps[:],
                                S_sb[:, k, :],
                                msg8[:, k, :, :].rearrange("p o i -> p (o i)"),
                                start=(k == 0), stop=(k == TPB - 1))
                        if pend is not None:
                            flush_block(*pend)
                        pend = (b, aggr_ps)
                    flush_block(*pend)
                    pend = None

                if step == 0:
                    _ph0_stack.close()
                if step < STEPS - 1:
                    ag_emit(hb_next, of_next)
                    out_full = of_next

              _ps.close()
              _hetp_stack.close()

              # ---------------- set2set pooling ----------------
              if os.environ.get("K_SKIP_S2S", "0") == "1":
                  with tc.tile_pool(name="ysk", bufs=1) as ysk:
                      y_skip = ysk.tile([P, GW], F32)
                      nc.gpsimd.memset(y_skip[:], 0.0)
                      nc.sync.dma_start(t_y.ap().rearrange("(w p) -> p w", p=P),
                                        y_skip[:])
              elif True:
               with (
                  tc.tile_pool(name="s2s", bufs=1) as s2s,
                  tc.tile_pool(name="s2w", bufs=2) as s2w,
                  tc.tile_pool(name="qgp", bufs=4) as qgp,
                  tc.tile_pool(name="lstp", bufs=1, space="PSUM") as lstp,
                  tc.tile_pool(name="qbp", bufs=2, space="PSUM") as qbp,
                  tc.tile_pool(name="gatp", bufs=2, space="PSUM") as gatp,
                  tc.tile_pool(name="rpp", bufs=1, space="PSUM") as rpp,
              ):
                  hh = s2s.tile([P, GW, D], F32)
                  cc = s2s.tile([P, GW, D], F32)
                  rp = s2s.tile([P, GW, D], F32)
                  hh16 = s2s.tile([P, GW, D], F16)
                  G_sb = s2s.tile([P, NB * GW, P], F16)
                  GT_sb = s2s.tile([P, NB * GW, P], F16)
                  wsb = s2s.tile([P, NB, 65], F16)
                  e_sb = s2s.tile([P, NB], F32)
                  ae_sb = s2s.tile([P, NB], F32)
                  l_hh = [s2s.tile([64, P], F16, name=f"lhh{w}", tag=f"lhh{w}")
                          for w in range(GW)]
                  l_rp = [s2s.tile([65, P], F16, name=f"lrp{w}", tag=f"lrp{w}")
                          for w in range(GW)]
                  nc.gpsimd.memset(hh[:], 0.0)
                  nc.gpsimd.memset(cc[:], 0.0)
                  nc.gpsimd.memset(rp[:], 0.0)
                  for w in range(GW):
                      nc.gpsimd.memset(l_rp[w][64:65, :], 1.0)
                  for w in range(GW):
                      nc.gpsimd.dma_start(
                          G_sb[:, w:NB * GW:GW, :],
                          t_G.ap()[w:NB * GW:GW].rearrange("t p q -> p t q"))
                  ghalf = (NB * GW + 1) // 2
                  nc.gpsimd.dma_start(
                      GT_sb[:, 0:ghalf, :],
                      t_GT.ap()[0:ghalf].rearrange("t p q -> p t q"))
                  nc.gpsimd.dma_start(
                      GT_sb[:, ghalf:NB * GW, :],
                      t_GT.ap()[ghalf:NB * GW].rearrange("t p q -> p t q"))
                  WL = {}
                  for nm, shp, dt in [("Wl1", [64, 256], F16),
                                      ("Wl2s", [65, 256], F16),
                                      ("Wla", [64, 64], F16),
                                      ("Wlb", [65, 64], F16),
                                      ("Wf", [64, 1], F16),
                                      ("b2col", [P, 1], F32)]:
                      WL[nm] = s2s.tile(shp, dt, tag=nm, name=nm)
                      nc.gpsimd.dma_start(WL[nm][:], wt[nm].ap())

                  def lstm_inputs(w_):
                      hhT_ps = lstp.tile([D, P], F32, tag="ltp")
                      nc.tensor.transpose(hhT_ps[:], hh[:, w_, :], ident32[:])
                      nc.vector.tensor_copy(l_hh[w_][:], hhT_ps[:])
                      rpT_ps = lstp.tile([D, P], F32, tag="ltp")
                      nc.tensor.transpose(rpT_ps[:], rp[:, w_, :], ident32[:])
                      nc.vector.tensor_copy(l_rp[w_][0:64, :], rpT_ps[:])

                  for it in range(STEPS):
                      if it == 0 and Z0:
                          # zero LSTM state + zero biases => q == 0 exactly,
                          # so softmax weights are uniform: wsb = [h | 1]
                          nc.vector.tensor_copy(wsb[:, :, 0:64], h_sb[:])
                          nc.gpsimd.memset(wsb[:, :, 64:65], 1.0)
                      else:
                        # --- LSTM update per graph window ---
                        for w_ in range(GW):
                          lstm_inputs(w_)
                          g_ps = gatp.tile([P, 256], F32, tag="gat")
                          nc.tensor.matmul(g_ps[:], l_hh[w_][:], WL["Wl1"][:],
                                           start=True, stop=False)
                          nc.tensor.matmul(g_ps[:], l_rp[w_][:], WL["Wl2s"][:],
                                           start=False, stop=True)
                          ig = s2w.tile([P, D], F16, tag="ig")
                          nc.scalar.activation(ig[:], g_ps[:, 0:64], Act.Sigmoid)
                          fg = s2w.tile([P, D], F16, tag="fg")
                          nc.scalar.activation(fg[:], g_ps[:, 64:128], Act.Sigmoid)
                          gg = s2w.tile([P, D], F16, tag="gg")
                          nc.scalar.activation(gg[:], g_ps[:, 128:192], Act.Tanh)
                          og = s2w.tile([P, D], F16, tag="og")
                          nc.scalar.activation(og[:], g_ps[:, 192:256], Act.Sigmoid)
                          t1 = s2w.tile([P, D], F32, tag="t1")
                          nc.vector.tensor_mul(t1[:], fg[:], cc[:, w_, :])
                          t2 = s2w.tile([P, D], F16, tag="t2")
                          nc.vector.tensor_mul(t2[:], ig[:], gg[:])
                          nc.vector.tensor_add(cc[:, w_, :], t1[:], t2[:])
                          tc_ = s2w.tile([P, D], F16, tag="tc")
                          nc.scalar.activation(tc_[:], cc[:, w_, :], Act.Tanh)
                          nc.vector.tensor_mul(hh[:, w_, :], og[:], tc_[:])
                          nc.vector.tensor_copy(hh16[:, w_, :], hh[:, w_, :])
                        # --- q[batch] via GT matmuls (PE), then batched e ---
                        qg = qgp.tile([P, NB, D], F16, tag="qg", bufs=2)
                        for b0 in range(0, NB, 8):
                          b1 = min(b0 + 8, NB)
                          qb_ps = qbp.tile([P, 8, D], F32, tag="qb")
                          for b in range(b0, b1):
                              for w_ in range(GW):
                                  nc.tensor.matmul(qb_ps[:, b - b0, :],
                                                   GT_sb[:, b * GW + w_, :],
                                                   hh16[:, w_, :],
                                                   start=(w_ == 0),
                                                   stop=(w_ == GW - 1))
                          nc.scalar.copy(qg[:, b0:b1, :], qb_ps[:, 0:b1 - b0, :])
                        ep_ = qgp.tile([P, NB, D], F16, tag="ep", bufs=2)
                        nc.vector.tensor_tensor(ep_[:], h_sb[:], qg[:], Alu.mult)
                        nc.vector.tensor_reduce(
                            e_sb[:], ep_[:],
                            axis=mybir.AxisListType.X, op=Alu.add)
                        nc.scalar.activation(ae_sb[:], e_sb[:], Act.Exp)
                        nc.vector.tensor_tensor(
                            wsb[:, :, 0:64], h_sb[:],
                            ae_sb.unsqueeze(2).broadcast_to([P, NB, D]),
                            Alu.mult)
                        nc.vector.tensor_copy(wsb[:, :, 64:65],
                                              ae_sb.unsqueeze(2))
                      # --- r_pool + asum via G matmuls ---
                      for w_ in range(GW):
                          rp_ps = rpp.tile([P, 65], F32, tag="rp")
                          for b in range(NB):
                              nc.tensor.matmul(rp_ps[:], G_sb[:, b * GW + w_, :],
                                               wsb[:, b, :],
                                               start=(b == 0), stop=(b == NB - 1))
                          asum = s2w.tile([P, 1], F32, tag="asum")
                          nc.vector.tensor_scalar_add(asum[:], rp_ps[:, 64:65],
                                                      1e-16)
                          rec = s2w.tile([P, 1], F32, tag="rec")
                          nc.vector.reciprocal(rec[:], asum[:])
                          nc.vector.tensor_scalar_mul(rp[:, w_, :],
                                                      rp_ps[:, 0:64], rec[:])

                  # --- final readout ---
                  y_sb = s2s.tile([P, GW], F32)
                  for w_ in range(GW):
                      lstm_inputs(w_)
                      t_ps = gatp.tile([P, 256], F32, tag="gat")
                      nc.tensor.matmul(t_ps[:, 0:64], l_hh[w_][:], WL["Wla"][:],
                                       start=True, stop=False)
                      nc.tensor.matmul(t_ps[:, 0:64], l_rp[w_][:], WL["Wlb"][:],
                                       start=False, stop=True)
                      t_sb = s2w.tile([P, D], F16, tag="tsb")
                      nc.scalar.activation(t_sb[:], t_ps[:, 0:64], Act.Relu)
                      tT_ps = lstp.tile([D, P], F16, tag="ttp")
                      nc.tensor.transpose(tT_ps[:], t_sb[:], ident[:])
                      tT_sb = s2w.tile([64, P], F16, tag="ttsb")
                      nc.vector.tensor_copy(tT_sb[:], tT_ps[:])
                      y_ps = rpp.tile([P, 1], F32, tag="yp")
                      nc.tensor.matmul(y_ps[:], tT_sb[:], WL["Wf"][:],
                                       start=True, stop=True)
                      nc.vector.tensor_scalar_add(y_sb[:, w_:w_ + 1], y_ps[:],
                                                  WL["b2col"][:])
                  nc.sync.dma_start(t_y.ap().rearrange("(w p) -> p w", p=P),
                                    y_sb[:])

    nc.compile()
    return nc


# ----------------------------------------------------------------------------
# Entry point
# ----------------------------------------------------------------------------

def kernel(**inputs):
    x = np.asarray(inputs["x"], np.float32)
    z = np.asarray(inputs["z"], np.float32)
    edge_attr = np.asarray(inputs["edge_attr"], np.float32)
    edge_index = np.asarray(inputs["edge_index"]).astype(np.int64)
    batch = np.asarray(inputs["batch"]).astype(np.int64)
    num_graphs = int(np.asarray(inputs["num_graphs"]))

    wts, b2e = _weights(*[inputs[k] for k in
                          ["lin0_w", "lin0_b", "emlp_w1", "emlp_b1", "emlp_w2",
                           "emlp_b2", "conv_root", "conv_bias", "gru_wi",
                           "gru_wh", "gru_bi", "gru_bh", "lstm_wi", "lstm_wh",
                           "lstm_bi", "lstm_bh", "lin1_w", "lin1_b", "lin2_w",
                           "lin2_b"]])
    assert np.all(b2e == 0.0), "nonzero emlp_b2 not supported"

    per_rank, meta = _preprocess(x, z, edge_attr, edge_index, batch, num_graphs)
    meta["Z0"] = bool(
        np.all(np.asarray(inputs["lstm_bi"], np.float32) == 0.0)
        and np.all(np.asarray(inputs["lstm_bh"], np.float32) == 0.0))
    nc = _build(meta)

    ident = np.eye(P, dtype=np.float16)
    ident32 = np.eye(P, dtype=np.float32)
    in_maps = []
    for r in range(NCORES):
        pr = per_rank[r]
        m = dict(xaT=pr["xaT"], eaT=pr["eaT"], idx=pr["idx"], S=pr["S"],
                 G=pr["G"], GT=pr["GT"], ident=ident, ident32=ident32,
                 **wts)
        in_maps.append(m)

    res = run_bass_kernel_spmd(nc, in_maps, core_ids=list(range(NCORES)))
    if res.exec_time_ns is not None:
        print(f"HW exec time: {res.exec_time_ns} ns")

    ys = []
    for r in range(NCORES):
        ys.append(res.results[r]["y_out"][:per_rank[r]["ngr"]])
    return np.concatenate(ys).astype(np.float32)



# revision 47
# speedup vs baseline: 1.0245x; 1.0137x over previous
"""Trainium2 Bass kernel for nn_Net_NNCONV (gnn_message_passing).

Strategy (8-core SPMD, data-parallel by graph):
 - Host: cut graphs into 8 contiguous shards (balanced node counts); within a
   shard, bin-pack nodes into NB 128-node blocks with <=256 in-edges per block
   (TPB=2 edge tiles per block); edges grouped by dst block in slot order.
 - fp16 everywhere on device (PE fp16 = 1 cycle/row vs 4 for f32; DVE 2x mode
   needs 2-byte packed operands).
 - Phase 0: lin0 -> h0 fp16; AllGather h0; he = relu(ea@W1+b1) fp16 in SBUF.
 - Step 0 fuses the ew = he@W2 production (PE matmuls + ACT psum->fp16 casts)
   into the message block loop: the fp16 ew tile is consumed directly and also
   written to DRAM for steps 1-4, which stream it back.
 - Per block (256 edges): indirect-gather gout fp16, in-place broadcast
   multiply (DVE 2x), in-place halving-tree partial reduce 64->8 over i,
   S-matmul (fp16 one-hot, 1/deg folded) with rhs [64o,8i] into PSUM
   accumulating the block's 2 tiles, single 1x reduce psum->aggr fp16.
 - GRU: per-block PE transposes + gh/gi matmuls (fp16), ACT psum copies,
   gate math batched over all blocks as [128, NB*64] fp16 DVE ops.
 - Set2Set: per-node q via fp16 GT one-hot matmuls on the (idle) PE; e =
   rowsum(h * q) as batched DVE mult+reduce (tensor_tensor_reduce crashes the
   device); r_pool/asum via one-hot G matmuls (G fp16, SBUF-resident); softmax
   skips max-subtraction (args O(10), fp32 exp).
 - ew_sb is triple-buffered so the 2 MB/block fabric-limited DMA is fully
   prefetched; the block psum reduce + GRU stages are software-pipelined one
   block behind the S-matmuls so DVE never waits on PE.
 - lin0/he preamble is fused into the step-0 pipeline (shared PSUM pools, no
   barrier); GRU gate math for the first block-half is folded into the block
   loop so only half remains serial before each AllGather.
 - Set2Set: iteration 0 is algebraic (zero LSTM biases => q==0 => uniform
   softmax), so it reduces to a mean-pool; q-copies batched 8 blocks/copy;
   G loaded w-major on the ACT queue so iter-0 r_pool starts early.
 - Measured bounds (trn2): DVE fp16 tensor_tensor caps at 2 elem/cyc/lane
   (mult 4.3us + tree 3.7us per 256-edge block ~= 440us/step floor); Pool
   tensor_tensor is ~7.6x slower AND contends with DVE for SBUF ports (never
   offload elementwise there); multi-column indirect_dma_start crashes the
   device; Shared collective outputs allow only one writer instruction.
"""
import contextlib
import os
import sys

sys.path.insert(0, "/opt/trn_rl_repo")

import numpy as np

import concourse.bass as bass
import concourse.tile as tile
from concourse import bacc, mybir
from concourse.bass import IndirectOffsetOnAxis
from concourse.bass_utils import run_bass_kernel_spmd

F32 = mybir.dt.float32
F16 = mybir.dt.float16
I32 = mybir.dt.int32
NCORES = 8
P = 128
D = 64
DD = D * D
W = 128
TPB = 2
EPB = TPB * P          # max edges per block
STEPS = 5
Alu = mybir.AluOpType
Act = mybir.ActivationFunctionType


# ----------------------------------------------------------------------------
# Host-side preprocessing
# ----------------------------------------------------------------------------

def _pack_rank(deg, lo, hi, NB):
    """Assign nodes [lo,hi) to NB blocks: <=128 nodes, <=EPB in-edges each.
    Returns (block, pos) per node or None if packing fails."""
    nodes = np.arange(lo, hi)
    order = nodes[np.argsort(-deg[nodes], kind="stable")]
    loads = np.zeros(NB, dtype=np.int64)
    counts = np.zeros(NB, dtype=np.int64)
    blk = np.zeros(hi - lo, dtype=np.int64)
    for nd in order:
        d = deg[nd]
        cand = np.flatnonzero(counts < P)
        if len(cand) == 0:
            return None
        bi = cand[np.argmin(loads[cand])]
        if loads[bi] + d > EPB:
            return None
        blk[nd - lo] = bi
        loads[bi] += d
        counts[bi] += 1
    pos = np.zeros(hi - lo, dtype=np.int64)
    ctr = np.zeros(NB, dtype=np.int64)
    for nd in order:
        b = blk[nd - lo]
        pos[nd - lo] = ctr[b]
        ctr[b] += 1
    return blk, pos


def _preprocess(x, z, edge_attr, edge_index, batch, num_graphs):
    N = x.shape[0]
    E = edge_index.shape[1]
    B = int(num_graphs)
    src = np.asarray(edge_index[0], dtype=np.int64)
    dst = np.asarray(edge_index[1], dtype=np.int64)
    batch = np.asarray(batch, dtype=np.int64)

    # --- shard cut: contiguous graphs, balanced node counts ---
    gcounts = np.bincount(batch, minlength=B)
    gcum = np.concatenate([[0], np.cumsum(gcounts)])
    targets = (np.arange(1, NCORES) * N) / NCORES
    cuts = np.searchsorted(gcum, targets)
    gcut = np.concatenate([[0], cuts, [B]])
    n0 = gcum[gcut]
    nodes_per = np.diff(n0)
    graphs_per = np.diff(gcut)

    GW = int(np.ceil(graphs_per.max() / P))

    deg = np.bincount(dst, minlength=N).astype(np.int64)
    dinv = (1.0 / np.maximum(deg, 1)).astype(np.float32)
    e_rank = np.searchsorted(n0[1:], dst, side="right")
    edges_per = np.bincount(e_rank, minlength=NCORES)

    NB = int(max(np.ceil(nodes_per.max() / P), np.ceil(edges_per.max() / EPB)))
    packs = None
    while packs is None:
        packs = []
        for r in range(NCORES):
            res = _pack_rank(deg, int(n0[r]), int(n0[r + 1]), NB)
            if res is None:
                packs = None
                NB += 1
                break
            packs.append(res)

    ET = NB * TPB
    SN = NB * P
    SLOTS = ET * P

    # AllGather is emitted in two block-halves; gathered rows live at
    # [rank-major half-1 | rank-major half-2] in out_full.
    H1B = NB // 2
    H1 = H1B * P
    H2 = (NB - H1B) * P

    node_block = np.zeros(N, dtype=np.int64)
    node_pos = np.zeros(N, dtype=np.int64)
    pad_global = np.zeros(N, dtype=np.int64)
    ag_global = np.zeros(N, dtype=np.int64)
    for r in range(NCORES):
        lo, hi = int(n0[r]), int(n0[r + 1])
        blk, pos = packs[r]
        node_block[lo:hi] = blk
        node_pos[lo:hi] = pos
        pad_global[lo:hi] = r * SN + blk * P + pos
        ag_global[lo:hi] = r * SN + pos * NB + blk

    xa = np.concatenate([np.asarray(x, np.float32),
                         np.asarray(z, np.float32)[:, None],
                         np.ones((N, 1), np.float32)], axis=1)  # [N, 17]

    per_rank = []
    for r in range(NCORES):
        lo, hi = int(n0[r]), int(n0[r + 1])
        g0 = int(gcut[r])
        ngr = int(gcut[r + 1] - gcut[r])

        xaT = np.zeros((17, SN), np.float16)
        loc = pad_global[lo:hi] - r * SN
        xaT[:, loc] = xa[lo:hi].T.astype(np.float16)

        em = e_rank == r
        es, ed = src[em], dst[em]
        eb = node_block[ed]
        eorder = np.argsort(eb, kind="stable")
        es, ed, eb = es[eorder], ed[eorder], eb[eorder]
        ea_r = np.asarray(edge_attr, np.float32)[em][eorder]

        eaT = np.zeros((6, SLOTS), np.float16)
        src_idx = np.zeros(SLOTS, np.int32)
        S = np.zeros((ET, P, P), np.float16)
        bstart = np.searchsorted(eb, np.arange(NB))
        bend = np.searchsorted(eb, np.arange(NB), side="right")
        for b in range(NB):
            cnt = bend[b] - bstart[b]
            assert cnt <= EPB, f"rank {r} block {b} edges {cnt} > {EPB}"
            sl = slice(bstart[b], bend[b])
            base = b * EPB
            eaT[:5, base:base + cnt] = ea_r[sl].T.astype(np.float16)
            eaT[5, base:base + cnt] = 1.0
            src_idx[base:base + cnt] = ag_global[es[sl]].astype(np.int32)
            vcol = node_pos[ed[sl]].astype(np.int64)
            vals = dinv[ed[sl]]
            within = np.arange(cnt)
            S[b * TPB + within // P, within % P, vcol] = vals.astype(np.float16)

        idx2d = src_idx.reshape(ET, P).T.copy()  # [128, ET]

        # pooling one-hot G[(b,w)][node_p, g_col] (+ transpose for q lookup)
        G = np.zeros((NB * GW, P, P), np.float16)
        gl = batch[lo:hi] - g0
        bb = node_block[lo:hi]
        pp = node_pos[lo:hi]
        ww = gl // P
        cc_ = gl % P
        G[bb * GW + ww, pp, cc_] = 1.0
        GT = np.ascontiguousarray(np.transpose(G, (0, 2, 1)))

        per_rank.append(dict(xaT=xaT, eaT=eaT, idx=idx2d, S=S, G=G, GT=GT,
                             ngr=ngr))

    meta = dict(NB=NB, SN=SN, GW=GW, ET=ET, SLOTS=SLOTS, H1B=H1B)
    return per_rank, meta


def _weights(lin0_w, lin0_b, emlp_w1, emlp_b1, emlp_w2, emlp_b2,
             conv_root, conv_bias, gru_wi, gru_wh, gru_bi, gru_bh,
             lstm_wi, lstm_wh, lstm_bi, lstm_bh, lin1_w, lin1_b, lin2_w, lin2_b):
    f = lambda a: np.asarray(a, np.float32)
    h = lambda a: np.asarray(a, np.float16)
    W0b = h(np.concatenate([f(lin0_w), f(lin0_b)[None, :]], 0))            # [17, 64]
    W1b = h(np.concatenate([f(emlp_w1), f(emlp_b1)[None, :]], 0))          # [6, 128]
    W2 = f(emlp_w2)
    b2e = f(emlp_b2)
    W2p = h(W2.reshape(W, D, D).transpose(0, 2, 1).reshape(W, D * D))      # [128, 4096]
    Wgh = np.zeros((65, 256), np.float16)
    Wgh[:64, 0:64] = h(conv_root)
    Wgh[:64, 64:256] = h(f(gru_wh).T)
    Wgh[64, 0:64] = h(conv_bias)
    Wgh[64, 64:256] = h(gru_bh)
    Wgi = h(np.concatenate([f(gru_wi).T, f(gru_bi)[None, :]], 0))          # [65, 192]
    Wl1 = h(f(lstm_wi).T[0:64, :] + f(lstm_wh).T)                          # [64, 256]
    Wl2s = h(np.concatenate([f(lstm_wi).T[64:128, :],
                             (f(lstm_bi) + f(lstm_bh))[None, :]], 0))      # [65, 256]
    Wla = h(f(lin1_w)[0:64, :])                                            # [64, 64]
    Wlb = h(np.concatenate([f(lin1_w)[64:128, :], f(lin1_b)[None, :]], 0))  # [65, 64]
    Wf = h(lin2_w)                                                         # [64, 1]
    b2col = np.full((P, 1), float(f(lin2_b).reshape(-1)[0]), np.float32)
    return dict(W0b=W0b, W1b=W1b, W2p=W2p, Wgh=Wgh, Wgi=Wgi, Wl1=Wl1,
                Wl2s=Wl2s, Wla=Wla, Wlb=Wlb, Wf=Wf, b2col=b2col), b2e


# ----------------------------------------------------------------------------
# Device kernel
# ----------------------------------------------------------------------------

def _build(meta):
    NB, SN, GW, ET, SLOTS = meta["NB"], meta["SN"], meta["GW"], meta["ET"], meta["SLOTS"]
    H1B = meta["H1B"]
    Z0 = meta["Z0"]
    AGN = NCORES * SN

    nc = bacc.Bacc("TRN2", target_bir_lowering=False, debug=False,
                   enable_asserts=False, num_devices=NCORES)
    t_xaT = nc.dram_tensor("xaT", [17, SN], F16, kind="ExternalInput")
    t_eaT = nc.dram_tensor("eaT", [6, SLOTS], F16, kind="ExternalInput")
    t_idx = nc.dram_tensor("idx", [P, ET], I32, kind="ExternalInput")
    t_S = nc.dram_tensor("S", [ET, P, P], F16, kind="ExternalInput")
    t_G = nc.dram_tensor("G", [NB * GW, P, P], F16, kind="ExternalInput")
    t_GT = nc.dram_tensor("GT", [NB * GW, P, P], F16, kind="ExternalInput")
    t_ident = nc.dram_tensor("ident", [P, P], F16, kind="ExternalInput")
    t_ident32 = nc.dram_tensor("ident32", [P, P], F32, kind="ExternalInput")
    wt = {}
    for nm, shp, dt in [("W0b", [17, 64], F16), ("W1b", [6, 128], F16),
                        ("W2p", [W, DD], F16), ("Wgh", [65, 256], F16),
                        ("Wgi", [65, 192], F16), ("Wl1", [64, 256], F16),
                        ("Wl2s", [65, 256], F16), ("Wla", [64, 64], F16),
                        ("Wlb", [65, 64], F16), ("Wf", [64, 1], F16),
                        ("b2col", [P, 1], F32)]:
        wt[nm] = nc.dram_tensor(nm, shp, dt, kind="ExternalInput")
    t_y = nc.dram_tensor("y_out", [GW * P], F32, kind="ExternalOutput")

    with nc.allow_low_precision("fp16 message-passing pipeline"), \
         tile.TileContext(nc) as tc:
        with (
            tc.tile_pool(name="persist", bufs=1) as pers,
            tc.tile_pool(name="dram", bufs=1, space="DRAM") as dram,
            tc.tile_pool(name="dram2", bufs=2, space="DRAM") as dram2,
        ):
            idx_sb = pers.tile([P, ET], I32)
            ident = pers.tile([P, P], F16)
            ident32 = pers.tile([P, P], F32)
            nc.sync.dma_start(idx_sb[:], t_idx.ap())
            nc.sync.dma_start(ident[:], t_ident.ap())
            nc.sync.dma_start(ident32[:], t_ident32.ap())

            h_sb = pers.tile([P, NB, D], F16)

            W2p_sb = pers.tile([W, DD], F16)
            Wgh_sb = pers.tile([65, 256], F16)
            Wgi_sb = pers.tile([65, 192], F16)
            nc.sync.dma_start(W2p_sb[:], wt["W2p"].ap())
            nc.sync.dma_start(Wgh_sb[:], wt["Wgh"].ap())
            nc.sync.dma_start(Wgi_sb[:], wt["Wgi"].ap())
            ghT_bufs = [pers.tile([65, P], F16, name=f"ghT{i}", tag=f"ghT{i}")
                        for i in range(3)]
            giT_bufs = [pers.tile([65, P], F16, name=f"giT{i}", tag=f"giT{i}")
                        for i in range(3)]
            for tl in ghT_bufs + giT_bufs:
                nc.gpsimd.memset(tl[64:65, :], 1.0)

            ew_dram = dram.tile([ET, P, DD], F16)


            def ag_alloc():
                hb_dram = dram2.tile([P, NB, D], F16, tag="hbd")
                out_full = dram2.tile([AGN, D], F16, tag="outf",
                                      addr_space="Shared")
                return hb_dram, out_full

            def ag_emit(hb_dram, out_full):
                nc.gpsimd.collective_compute(
                    "AllGather", Alu.bypass,
                    replica_groups=[list(range(NCORES))],
                    ins=[hb_dram.opt()], outs=[out_full.opt()])

            def emit_allgather():
                hb_dram, out_full = ag_alloc()
                nc.scalar.dma_start(hb_dram[:], h_sb[:])
                ag_emit(hb_dram, out_full)
                return out_full

            # ---------------- message passing steps ----------------
            with tc.tile_pool(name="ms", bufs=2) as mssb:
              _hetp_stack = contextlib.ExitStack()
              hetp = _hetp_stack.enter_context(tc.tile_pool(name="het", bufs=1))
              _ph0_stack = contextlib.ExitStack()
              ph0 = _ph0_stack.enter_context(tc.tile_pool(name="ph0", bufs=1))
              heT = ph0.tile([W, SLOTS], F16)
              _ps = contextlib.ExitStack()
              aggrp = _ps.enter_context(
                  tc.tile_pool(name="aggrp", bufs=2, space="PSUM"))
              ewpsp = _ps.enter_context(
                  tc.tile_pool(name="ewp", bufs=2, space="PSUM"))
              tpp = _ps.enter_context(
                  tc.tile_pool(name="tpp", bufs=1, space="PSUM"))
              gpp = _ps.enter_context(
                  tc.tile_pool(name="gpp", bufs=1, space="PSUM"))
              CH = 8  # GRU interleave chunk (blocks)

              # ---- lin0 + he preamble, fused into the step-0 pipeline ----
              # (shares aggrp/ewpsp PSUM banks; no barrier so the tile
              # scheduler overlaps he production with step 0's block loop)
              xaT_sb = ph0.tile([17, SN], F16)
              W0b_sb = ph0.tile([17, 64], F16)
              eaT_sb = ph0.tile([6, SLOTS], F16)
              W1b_sb = ph0.tile([6, 128], F16)
              nc.gpsimd.dma_start(xaT_sb[:], t_xaT.ap())
              nc.gpsimd.dma_start(W0b_sb[:], wt["W0b"].ap())
              nc.gpsimd.dma_start(eaT_sb[:], t_eaT.ap())
              nc.gpsimd.dma_start(W1b_sb[:], wt["W1b"].ap())
              for b in range(NB):
                  h0ps = aggrp.tile([P, D], F32, tag="aggr")
                  nc.tensor.matmul(h0ps[:], xaT_sb[:, b * P:(b + 1) * P],
                                   W0b_sb[:], start=True, stop=True)
                  nc.scalar.activation(h_sb[:, b, :], h0ps[:], Act.Relu)
              out_full = emit_allgather()
              for t in range(ET):
                  heps_t = ewpsp.tile([W, P], F32, tag="ewps")
                  nc.tensor.matmul(heps_t[:], W1b_sb[:],
                                   eaT_sb[:, t * P:(t + 1) * P],
                                   start=True, stop=True)
                  nc.scalar.activation(heT[:, t * P:(t + 1) * P],
                                       heps_t[:], Act.Relu)

              # per-chunk GRU intermediates: small cycling tiles instead of
              # NB-sized arrays (frees ~47KB/partition for deeper ew buffers)
              cur = {}

              def gru_stage_a(b):
                  # gh = [h|1] @ Wgh for block b (uses h from previous step)
                  hT_ps = tpp.tile([D, P], F16, tag="tp")
                  nc.tensor.transpose(hT_ps[:], h_sb[:, b, :], ident[:])
                  ghT = ghT_bufs[b % 3]
                  nc.scalar.copy(ghT[0:64, :], hT_ps[:])
                  gh_ps = gpp.tile([P, 512], F32, tag="ghps")
                  nc.tensor.matmul(gh_ps[:, 0:256], ghT[:], Wgh_sb[:],
                                   start=True, stop=True)
                  nc.scalar.copy(cur["gh"][:, b - cur["c0"], :],
                                 gh_ps[:, 0:256])

              hb_cell = [None]

              def flush_block(b, aggr_ps):
                  if b % CH == 0:
                      cur["c0"] = b
                      cur["gh"] = hetp.tile([P, CH, 256], F16, tag="ghc",
                                            bufs=2, name="ghc")
                      cur["gi"] = hetp.tile([P, CH, 192], F16, tag="gic",
                                            bufs=2, name="gic")
                      cur["ag"] = hetp.tile([P, CH, D], F16, tag="agc",
                                            bufs=2, name="agc")
                      cur["m"] = hetp.tile([P, CH, D], F16, tag="mc",
                                           bufs=2, name="mc")
                      cur["g1"] = hetp.tile([P, CH, D], F16, tag="g1c",
                                            bufs=2, name="g1c")
                      cur["g2"] = hetp.tile([P, CH, D], F16, tag="g2c",
                                            bufs=2, name="g2c")
                  nc.vector.tensor_reduce(
                      cur["ag"][:, b - cur["c0"], :],
                      aggr_ps.rearrange("p (o i) -> p o i", o=D),
                      axis=mybir.AxisListType.X, op=Alu.add)
                  gru_stage_a(b)
                  if b % CH == CH - 1 or b == NB - 1:
                      c0 = b - b % CH
                      gru_stage_b(c0, b + 1)
                      # h for this chunk is final: gate math + stage to DRAM
                      # now so the AllGather launches right after the loop
                      gate_math(c0, b + 1)
                      if hb_cell[0] is not None:
                          nc.scalar.dma_start(hb_cell[0][:, c0:b + 1, :],
                                              h_sb[:, c0:b + 1, :])

              def gate_math(c0, c1):
                  # GRU batched gate math (torch order: r, z, n)
                  n_ = c1 - c0
                  g1h = cur["g1"][:, 0:n_, :]
                  g2h = cur["g2"][:, 0:n_, :]
                  hh_ = h_sb[:, c0:c1, :]
                  gih = cur["gi"][:, 0:n_, :]
                  ghh = cur["gh"][:, 0:n_, :]
                  nc.vector.tensor_tensor(g1h, gih[:, :, 0:64],
                                          ghh[:, :, 64:128], Alu.add)
                  nc.scalar.activation(g1h, g1h, Act.Sigmoid)   # r
                  nc.vector.tensor_tensor(g2h, gih[:, :, 64:128],
                                          ghh[:, :, 128:192], Alu.add)
                  nc.scalar.activation(g2h, g2h, Act.Sigmoid)   # z
                  ngv = gih[:, :, 128:192]
                  nc.vector.tensor_tensor(ghh[:, :, 192:256], g1h,
                                          ghh[:, :, 192:256], Alu.mult)
                  nc.vector.tensor_tensor(ngv, ngv,
                                          ghh[:, :, 192:256], Alu.add)
                  nc.scalar.activation(ngv, ngv, Act.Tanh)          # n
                  nc.vector.tensor_tensor(g1h, hh_, ngv, Alu.subtract)
                  nc.vector.tensor_tensor(g1h, g2h, g1h, Alu.mult)
                  nc.vector.tensor_tensor(hh_, ngv, g1h, Alu.add)

              step_cell = [0]

              def gru_stage_b(c0, c1):
                  # m_pre for chunk, then gi = [relu(m)|1] @ Wgi per block
                  n_ = c1 - c0
                  nc.vector.tensor_tensor(cur["m"][:, 0:n_, :],
                                          cur["ag"][:, 0:n_, :],
                                          cur["gh"][:, 0:n_, 0:64], Alu.add)
                  for b in range(c0, c1):
                      mT_ps = tpp.tile([D, P], F16, tag="tp")
                      nc.tensor.transpose(mT_ps[:], cur["m"][:, b - c0, :],
                                          ident[:])
                      giT = giT_bufs[b % 3]
                      nc.scalar.activation(giT[0:64, :], mT_ps[:], Act.Relu)
                      if step_cell[0] == 0:
                          gi_ps = gpp.tile([P, 512], F32, tag="ghps")
                      else:
                          # ewpsp banks are idle after step 0: 2-deep
                          # pipelining for the gi chain instead of 1
                          gi_ps = ewpsp.tile([P, 512], F32, tag="ewps")
                      nc.tensor.matmul(gi_ps[:, 0:192], giT[:], Wgi_sb[:],
                                       start=True, stop=True)
                      nc.scalar.copy(cur["gi"][:, b - c0, :], gi_ps[:, 0:192])

              for step in range(STEPS):
                step_cell[0] = step
                if step < STEPS - 1:
                    hb_next, of_next = ag_alloc()
                    hb_cell[0] = hb_next
                else:
                    hb_cell[0] = None
                # --- block loop: messages + aggregation, GRU interleaved ---
                if True:
                    pend = None
                    # gathers issued GLA blocks ahead so they clear the DMA
                    # rings before DVE needs them
                    GLA = 3
                    gouts = {}

                    def issue_gather(bb):
                        g = mssb.tile([P, TPB, D], F16, tag="gout", bufs=6,
                                      name="gout")
                        for k in range(TPB):
                            nc.gpsimd.indirect_dma_start(
                                out=g[:, k, :], out_offset=None,
                                in_=out_full[:],
                                in_offset=IndirectOffsetOnAxis(
                                    ap=idx_sb[:, bb * TPB + k:bb * TPB + k + 1],
                                    axis=0))
                        gouts[bb] = g

                    for bb in range(min(GLA, NB)):
                        issue_gather(bb)
                    for b in range(NB):
                        t0 = b * TPB
                        ew_sb = mssb.tile([P, TPB, DD], F16, tag="ewsb",
                                          bufs=3)
                        if step == 0:
                            for k in range(TPB):
                                for q in range(4):
                                    ewps = ewpsp.tile([P, 1024], F32, tag="ewps")
                                    for hf in range(2):
                                        c0 = q * 1024 + hf * 512
                                        nc.tensor.matmul(
                                            ewps[:, hf * 512:(hf + 1) * 512],
                                            heT[:, (t0 + k) * P:(t0 + k + 1) * P],
                                            W2p_sb[:, c0:c0 + 512],
                                            start=True, stop=True)
                                    nc.scalar.copy(
                                        ew_sb[:, k, q * 1024:(q + 1) * 1024],
                                        ewps[:])
                                nc.sync.dma_start(
                                    ew_dram[t0 + k, :, :], ew_sb[:, k, :])
                        else:
                            for k in range(TPB):
                                nc.sync.dma_start(
                                    ew_sb[:, k, :],
                                    ew_dram[t0 + k, :, :])
                        S_sb = mssb.tile([P, TPB, P], F16, tag="S", bufs=4)
                        nc.sync.dma_start(
                            S_sb[:], t_S.ap()[t0:t0 + TPB].rearrange(
                                "t e v -> e t v"))
                        if b + GLA < NB:
                            issue_gather(b + GLA)
                        gout = gouts.pop(b)
                        # in-place broadcast multiply + halving tree over i
                        ew4 = ew_sb.rearrange("p t (o i) -> p t o i", o=D)
                        msg8 = mssb.tile([P, TPB, D, 8], F16, tag="msg8")
                        # per-tile mults: tile 0's multiply starts as soon
                        # as its 1MB of ew lands (step 0: also overlaps the
                        # ew_dram write drain)
                        for k in range(TPB):
                            nc.vector.tensor_tensor(
                                ew4[:, k, :, :], ew4[:, k, :, :],
                                gout[:, k, :].unsqueeze(1)
                                    .broadcast_to([P, D, D]),
                                Alu.mult)
                        for half in (32, 16):
                            nc.vector.tensor_tensor(
                                ew4[:, :, :, 0:half], ew4[:, :, :, 0:half],
                                ew4[:, :, :, half:2 * half], Alu.add)
                        nc.vector.tensor_tensor(
                            msg8[:], ew4[:, :, :, 0:8],
                            ew4[:, :, :, 8:16], Alu.add)
                        aggr_ps = aggrp.tile([P, D * 8], F32, tag="aggr")
                        for k in range(TPB):
                            nc.tensor.matmul(
                                aggr_ps[:],
                                S_sb[:, k, :],
                                msg8[:, k, :, :].rearrange("p o i -> p (o i)"),
                                start=(k == 0), stop=(k == TPB - 1))
                        if pend is not None:
                            flush_block(*pend)
                        pend = (b, aggr_ps)
                    flush_block(*pend)
                    pend = None

                if step == 0:
                    _ph0_stack.close()
                if step < STEPS - 1:
                    ag_emit(hb_next, of_next)
                    out_full = of_next

              _ps.close()
              _hetp_stack.close()

              # ---------------- set2set pooling ----------------
              if os.environ.get("K_SKIP_S2S", "0") == "1":
                  with tc.tile_pool(name="ysk", bufs=1) as ysk:
                      y_skip = ysk.tile([P, GW], F32)
                      nc.gpsimd.memset(y_skip[:], 0.0)
                      nc.sync.dma_start(t_y.ap().rearrange("(w p) -> p w", p=P),
                                        y_skip[:])
              elif True:
               with (
                  tc.tile_pool(name="s2s", bufs=1) as s2s,
                  tc.tile_pool(name="s2w", bufs=2) as s2w,
                  tc.tile_pool(name="qgp", bufs=4) as qgp,
                  tc.tile_pool(name="lstp", bufs=1, space="PSUM") as lstp,
                  tc.tile_pool(name="qbp", bufs=2, space="PSUM") as qbp,
                  tc.tile_pool(name="gatp", bufs=2, space="PSUM") as gatp,
                  tc.tile_pool(name="rpp", bufs=1, space="PSUM") as rpp,
              ):
                  hh = s2s.tile([P, GW, D], F32)
                  cc = s2s.tile([P, GW, D], F32)
                  rp = s2s.tile([P, GW, D], F32)
                  hh16 = s2s.tile([P, GW, D], F16)
                  G_sb = s2s.tile([P, NB * GW, P], F16)
                  GT_sb = s2s.tile([P, NB * GW, P], F16)
                  wsb = s2s.tile([P, NB, 65], F16)
                  e_sb = s2s.tile([P, NB], F32)
                  ae_sb = s2s.tile([P, NB], F32)
                  l_hh = [s2s.tile([64, P], F16, name=f"lhh{w}", tag=f"lhh{w}")
                          for w in range(GW)]
                  l_rp = [s2s.tile([65, P], F16, name=f"lrp{w}", tag=f"lrp{w}")
                          for w in range(GW)]
                  nc.gpsimd.memset(hh[:], 0.0)
                  nc.gpsimd.memset(cc[:], 0.0)
                  nc.gpsimd.memset(rp[:], 0.0)
                  for w in range(GW):
                      nc.gpsimd.memset(l_rp[w][64:65, :], 1.0)
                  for w in range(GW):
                      nc.gpsimd.dma_start(
                          G_sb[:, w:NB * GW:GW, :],
                          t_G.ap()[w:NB * GW:GW].rearrange("t p q -> p t q"))
                  ghalf = (NB * GW + 1) // 2
                  nc.scalar.dma_start(
                      GT_sb[:, 0:ghalf, :],
                      t_GT.ap()[0:ghalf].rearrange("t p q -> p t q"))
                  nc.scalar.dma_start(
                      GT_sb[:, ghalf:NB * GW, :],
                      t_GT.ap()[ghalf:NB * GW].rearrange("t p q -> p t q"))
                  WL = {}
                  for nm, shp, dt in [("Wl1", [64, 256], F16),
                                      ("Wl2s", [65, 256], F16),
                                      ("Wla", [64, 64], F16),
                                      ("Wlb", [65, 64], F16),
                                      ("Wf", [64, 1], F16),
                                      ("b2col", [P, 1], F32)]:
                      WL[nm] = s2s.tile(shp, dt, tag=nm, name=nm)
                      nc.gpsimd.dma_start(WL[nm][:], wt[nm].ap())

                  def lstm_inputs(w_):
                      hhT_ps = lstp.tile([D, P], F32, tag="ltp")
                      nc.tensor.transpose(hhT_ps[:], hh[:, w_, :], ident32[:])
                      nc.vector.tensor_copy(l_hh[w_][:], hhT_ps[:])
                      rpT_ps = lstp.tile([D, P], F32, tag="ltp")
                      nc.tensor.transpose(rpT_ps[:], rp[:, w_, :], ident32[:])
                      nc.vector.tensor_copy(l_rp[w_][0:64, :], rpT_ps[:])

                  for it in range(STEPS):
                      if it == 0 and Z0:
                          # zero LSTM state + zero biases => q == 0 exactly,
                          # so softmax weights are uniform: wsb = [h | 1]
                          nc.vector.tensor_copy(wsb[:, :, 0:64], h_sb[:])
                          nc.gpsimd.memset(wsb[:, :, 64:65], 1.0)
                      else:
                        # --- LSTM update per graph window ---
                        for w_ in range(GW):
                          lstm_inputs(w_)
                          g_ps = gatp.tile([P, 256], F32, tag="gat")
                          nc.tensor.matmul(g_ps[:], l_hh[w_][:], WL["Wl1"][:],
                                           start=True, stop=False)
                          nc.tensor.matmul(g_ps[:], l_rp[w_][:], WL["Wl2s"][:],
                                           start=False, stop=True)
                          ig = s2w.tile([P, D], F16, tag="ig")
                          nc.scalar.activation(ig[:], g_ps[:, 0:64], Act.Sigmoid)
                          fg = s2w.tile([P, D], F16, tag="fg")
                          nc.scalar.activation(fg[:], g_ps[:, 64:128], Act.Sigmoid)
                          gg = s2w.tile([P, D], F16, tag="gg")
                          nc.scalar.activation(gg[:], g_ps[:, 128:192], Act.Tanh)
                          og = s2w.tile([P, D], F16, tag="og")
                          nc.scalar.activation(og[:], g_ps[:, 192:256], Act.Sigmoid)
                          t1 = s2w.tile([P, D], F32, tag="t1")
                          nc.vector.tensor_mul(t1[:], fg[:], cc[:, w_, :])
                          t2 = s2w.tile([P, D], F16, tag="t2")
                          nc.vector.tensor_mul(t2[:], ig[:], gg[:])
                          nc.vector.tensor_add(cc[:, w_, :], t1[:], t2[:])
                          tc_ = s2w.tile([P, D], F16, tag="tc")
                          nc.scalar.activation(tc_[:], cc[:, w_, :], Act.Tanh)
                          nc.vector.tensor_mul(hh[:, w_, :], og[:], tc_[:])
                          nc.vector.tensor_copy(hh16[:, w_, :], hh[:, w_, :])
                        # --- q[batch] via GT matmuls (PE), then batched e ---
                        qg = qgp.tile([P, NB, D], F16, tag="qg", bufs=2)
                        for b0 in range(0, NB, 8):
                          b1 = min(b0 + 8, NB)
                          qb_ps = qbp.tile([P, 8, D], F32, tag="qb")
                          for b in range(b0, b1):
                              for w_ in range(GW):
                                  nc.tensor.matmul(qb_ps[:, b - b0, :],
                                                   GT_sb[:, b * GW + w_, :],
                                                   hh16[:, w_, :],
                                                   start=(w_ == 0),
                                                   stop=(w_ == GW - 1))
                          nc.scalar.copy(qg[:, b0:b1, :], qb_ps[:, 0:b1 - b0, :])
                        ep_ = qgp.tile([P, NB, D], F16, tag="ep", bufs=2)
                        nc.vector.tensor_tensor(ep_[:], h_sb[:], qg[:], Alu.mult)
                        nc.vector.tensor_reduce(
                            e_sb[:], ep_[:],
                            axis=mybir.AxisListType.X, op=Alu.add)
                        nc.scalar.activation(ae_sb[:], e_sb[:], Act.Exp)
                        nc.vector.tensor_tensor(
                            wsb[:, :, 0:64], h_sb[:],
                            ae_sb.unsqueeze(2).broadcast_to([P, NB, D]),
                            Alu.mult)
                        nc.vector.tensor_copy(wsb[:, :, 64:65],
                                              ae_sb.unsqueeze(2))
                      # --- r_pool + asum via G matmuls ---
                      for w_ in range(GW):
                          rp_ps = rpp.tile([P, 65], F32, tag="rp")
                          for b in range(NB):
                              nc.tensor.matmul(rp_ps[:], G_sb[:, b * GW + w_, :],
                                               wsb[:, b, :],
                                               start=(b == 0), stop=(b == NB - 1))
                          asum = s2w.tile([P, 1], F32, tag="asum")
                          nc.vector.tensor_scalar_add(asum[:], rp_ps[:, 64:65],
                                                      1e-16)
                          rec = s2w.tile([P, 1], F32, tag="rec")
                          nc.vector.reciprocal(rec[:], asum[:])
                          nc.vector.tensor_scalar_mul(rp[:, w_, :],
                                                      rp_ps[:, 0:64], rec[:])

                  # --- final readout ---
                  y_sb = s2s.tile([P, GW], F32)
                  for w_ in range(GW):
                      lstm_inputs(w_)
                      t_ps = gatp.tile([P, 256], F32, tag="gat")
                      nc.tensor.matmul(t_ps[:, 0:64], l_hh[w_][:], WL["Wla"][:],
                                       start=True, stop=False)
                      nc.tensor.matmul(t_ps[:, 0:64], l_rp[w_][:], WL["Wlb"][:],
                                       start=False, stop=True)
                      t_sb = s2w.tile([P, D], F16, tag="tsb")
                      nc.scalar.activation(t_sb[:], t_ps[:, 0:64], Act.Relu)
                      tT_ps = lstp.tile([D, P], F16, tag="ttp")
                      nc.tensor.transpose(tT_ps[:], t_sb[:], ident[:])
                      tT_sb = s2w.tile([64, P], F16, tag="ttsb")
                      nc.vector.tensor_copy(tT_sb[:], tT_ps[:])
                      y_ps = rpp.tile([P, 1], F32, tag="yp")
                      nc.tensor.matmul(y_ps[:], tT_sb[:], WL["Wf"][:],
                                       start=True, stop=True)
                      nc.vector.tensor_scalar_add(y_sb[:, w_:w_ + 1], y_ps[:],
                                                  WL["b2col"][:])
                  nc.sync.dma_start(t_y.ap().rearrange("(w p) -> p w", p=P),
                                    y_sb[:])

    nc.compile()
    return nc


# ----------------------------------------------------------------------------
# Entry point
# ----------------------------------------------------------------------------

def kernel(**inputs):
    x = np.asarray(inputs["x"], np.float32)
    z = np.asarray(inputs["z"], np.float32)
    edge_attr = np.asarray(inputs["edge_attr"], np.float32)
    edge_index = np.asarray(inputs["edge_index"]).astype(np.int64)
    batch = np.asarray(inputs["batch"]).astype(np.int64)
    num_graphs = int(np.asarray(inputs["num_graphs"]))

    wts, b2e = _weights(*[inputs[k] for k in
                          ["lin0_w", "lin0_b", "emlp_w1", "emlp_b1", "emlp_w2",
                           "emlp_b2", "conv_root", "conv_bias", "gru_wi",
                           "gru_wh", "gru_bi", "gru_bh", "lstm_wi", "lstm_wh",
                           "lstm_bi", "lstm_bh", "lin1_w", "lin1_b", "lin2_w",
                           "lin2_b"]])
    assert np.all(b2e == 0.0), "nonzero emlp_b2 not supported"

    per_rank, meta = _preprocess(x, z, edge_attr, edge_index, batch, num_graphs)
    meta["Z0"] = bool(
        np.all(np.asarray(inputs["lstm_bi"], np.float32) == 0.0)
        and np.all(np.asarray(inputs["lstm_bh"], np.float32) == 0.0))
    nc = _build(meta)

    ident = np.eye(P, dtype=np.float16)
    ident32 = np.eye(P, dtype=np.float32)
    in_maps = []
    for r in range(NCORES):
        pr = per_rank[r]
        m = dict(xaT=pr["xaT"], eaT=pr["eaT"], idx=pr["idx"], S=pr["S"],
                 G=pr["G"], GT=pr["GT"], ident=ident, ident32=ident32,
                 **wts)
        in_maps.append(m)

    res = run_bass_kernel_spmd(nc, in_maps, core_ids=list(range(NCORES)))
    if res.exec_time_ns is not None:
        print(f"HW exec time: {res.exec_time_ns} ns")

    ys = []
    for r in range(NCORES):
        ys.append(res.results[r]["y_out"][:per_rank[r]["ngr"]])
    return np.concatenate(ys).astype(np.float32)

